# revision 11
# baseline (speedup 1.0000x reference)
"""Distributed Trainium2 kernel for nn_Attention (dense transformer block:
fused QKV projection + per-head RMSNorm + rotary + causal GQA attention + output
projection), running SPMD on 8 NeuronCores.

Sharding (rank-uniform, no divergent control flow):
  - 8 cores = 2 batch groups x 4 tensor-parallel ranks.
  - Core c: batch b = c // 4, rank r = c % 4.
  - QKV projection + attention are head-sharded: core r computes q heads
    4r..4r+3 and kv head r for ALL tokens of its batch.
  - Per-head AllGather re-shards y from head-split to token-split, overlapped
    with the next head's attention; the output projection then runs locally
    with the full contraction dim in 4 per-head passes (no all-reduce).

Layout tricks:
  - Host pre-transposes x, wqkv, wo so the kernel's matmuls need no on-device
    transposes (except tiny 128x128 PE transposes for V).
  - Rope's even/odd pair swap is a 128x128 permutation matmul in bf16
    (cheap; fp32 matmuls run at quarter rate).
  - Scores are computed transposed [kv, q]; exp is fused into the PSUM->SBUF
    eviction on the ScalarEngine, batched two kv-blocks per activation to
    amortize the ~300-cycle instruction overhead.
  - The softmax denominator is accumulated on the GpSimd engine (tensor_add
    over the exp tiles) and reduced across partitions with
    partition_all_reduce -- zero TensorEngine cost.
  - RMSNorm's sum-of-squares also uses partition_all_reduce instead of a
    ones-vector matmul; the 1/sqrt(head_dim) score scale folds into the
    q-side scalar.
  - All big matmuls run in bf16 with f32 PSUM accumulation.
"""

import numpy as np
import ml_dtypes

import concourse.bass as bass
import concourse.bass_isa as bass_isa
import concourse.mybir as mybir
import concourse.tile as tile
from concourse import bacc
from concourse.bass_utils import run_bass_kernel_spmd

BF16 = mybir.dt.bfloat16
F32 = mybir.dt.float32

DIM = 2048
NH = 16
NKV = 4
HD = 128
EPS = 1e-5
N_CORES = 8
RG = [[0, 1, 2, 3], [4, 5, 6, 7]]  # per-batch tensor-parallel groups

HL = NH // NKV  # q heads per core (= GQA group size) = 4
EW = HL * HD + 2 * HD  # wqkv column-slice width per core = 768
NDT = DIM // 128  # contraction tiles = 16


def build_graph(S):
    """Build + compile the SPMD graph for sequence length S. Returns nc."""
    TPT = S // 4       # tokens per core after the gather (output rows per core)
    TCW = S // 4       # token chunk width for phase 1 (moving dim <= 512)
    NTT = S // TCW     # number of token chunks = 4
    QC = 512           # attention q-chunk width
    KB = 128           # kv block size
    NQC = S // QC      # q chunks per head
    NB = S // 128      # 128-token blocks (for V layout)
    AVDEPTH = 3        # kv-block pairs the AV matmuls trail the score matmuls

    nc = bacc.Bacc("TRN2", target_bir_lowering=False, debug=False,
                   num_devices=N_CORES)

    # ---- DRAM I/O ----
    xT_d = nc.dram_tensor("xT", [DIM, S], BF16, kind="ExternalInput")
    w_d = nc.dram_tensor("wslice", [DIM, EW], BF16, kind="ExternalInput")
    wo_d = nc.dram_tensor("woT", [DIM, DIM], BF16, kind="ExternalInput")
    cos_d = nc.dram_tensor("cosF", [128, S], F32, kind="ExternalInput")
    sin_d = nc.dram_tensor("sinF", [128, S], F32, kind="ExternalInput")
    swp_d = nc.dram_tensor("swapP", [128, 128], BF16, kind="ExternalInput")
    idn_d = nc.dram_tensor("ident", [128, 128], BF16, kind="ExternalInput")
    msk_d = nc.dram_tensor("masks", [KB, (QC // KB) * QC], BF16, kind="ExternalInput")
    qw_d = nc.dram_tensor("qw", [128, 1], F32, kind="ExternalInput")
    kw_d = nc.dram_tensor("kw", [128, 1], F32, kind="ExternalInput")
    out_d = nc.dram_tensor("out", [DIM, TPT], BF16, kind="ExternalOutput")

    with tile.TileContext(nc) as tc:
        with tc.tile_pool(name="const", bufs=1) as cpool, \
             tc.tile_pool(name="big", bufs=1) as bigpool, \
             tc.tile_pool(name="dram", bufs=1, space="DRAM") as dpool:

            # constants
            swp = cpool.tile([128, 128], BF16, tag="swp")
            nc.sync.dma_start(swp[:], swp_d[:])
            idn = cpool.tile([128, 128], BF16, tag="idn")
            nc.sync.dma_start(idn[:], idn_d[:])
            msk = cpool.tile([KB, (QC // KB) * QC], BF16, tag="msk")
            nc.sync.dma_start(msk[:], msk_d[:])
            qw = cpool.tile([128, 1], F32, tag="qw")
            nc.sync.dma_start(qw[:], qw_d[:])
            kw = cpool.tile([128, 1], F32, tag="kw")
            nc.sync.dma_start(kw[:], kw_d[:])
            b0 = cpool.tile([128, 1], F32, tag="b0")
            nc.vector.memset(b0[:], 0.0)
            bq = cpool.tile([1, 1], F32, tag="bq")
            nc.vector.memset(bq[:], float(HD * EPS))
            bk = cpool.tile([1, 1], F32, tag="bk")
            nc.vector.memset(bk[:], float(EPS))

            # long-lived activations
            qT = bigpool.tile([128, HL * S], BF16, tag="qT")
            kT = bigpool.tile([128, S], BF16, tag="kT")
            V = bigpool.tile([128, S], BF16, tag="V")   # [tok%128, blk*128+d]

            # ---------------- Phase 1: QKV + norm + rope ----------------
            with tc.tile_pool(name="wq", bufs=1) as wpool, \
                 tc.tile_pool(name="x", bufs=2) as xpool, \
                 tc.tile_pool(name="cs", bufs=2) as cspool, \
                 tc.tile_pool(name="scr", bufs=2) as scr, \
                 tc.tile_pool(name="smol", bufs=2) as smol, \
                 tc.tile_pool(name="p1", bufs=2, space="PSUM") as p1, \
                 tc.tile_pool(name="psw", bufs=2, space="PSUM") as psw, \
                 tc.tile_pool(name="pvt", bufs=2, space="PSUM") as pvt:

                # full wqkv slice, staged once: [128, dt*EW + e]
                w_sb = wpool.tile([128, NDT * EW], BF16, tag="w")

                def process_qk(ps, et, tt, cos_t, sin_t):
                    is_q = et < HL
                    # sum of squares over head_dim via gpsimd partition reduce
                    sqv = smol.tile([128, TCW], F32, tag="sq2", name="sqv")
                    nc.scalar.activation(
                        sqv[:], ps[:],
                        mybir.ActivationFunctionType.Square, bias=b0[:])
                    ssb = scr.tile([128, TCW], F32, tag="ssb", name="ssb")
                    nc.gpsimd.partition_all_reduce(
                        ssb[:], sqv[:], channels=128,
                        reduce_op=bass_isa.ReduceOp.add)
                    sq = smol.tile([1, TCW], F32, tag="sqs", name="sq")
                    if is_q:
                        # 1/sqrt(ss + HD*eps) folds the 1/sqrt(HD) score scale
                        nc.scalar.activation(
                            sq[:], ssb[0:1, :],
                            mybir.ActivationFunctionType.Sqrt,
                            bias=bq[:], scale=1.0)
                    else:
                        nc.scalar.activation(
                            sq[:], ssb[0:1, :],
                            mybir.ActivationFunctionType.Sqrt,
                            bias=bk[:], scale=1.0 / HD)
                    inv = smol.tile([1, TCW], F32, tag="inv", name="inv")
                    nc.vector.reciprocal_approx_fast(inv[:], sq[:])
                    invb = scr.tile([128, TCW], F32, tag="invb", name="invb")
                    nc.gpsimd.partition_broadcast(invb[:], inv[:])
                    qf = scr.tile([128, TCW], BF16, tag="qf", name="qf")
                    nc.scalar.mul(qf[:], ps[:], (qw if is_q else kw)[:])
                    # rope: pair swap via bf16 permutation matmul, sinF signed
                    sw = psw.tile([128, TCW], F32, tag="sw", name="sw")
                    nc.tensor.matmul(sw[:], swp[:], qf[:],
                                     start=True, stop=True)
                    t1 = scr.tile([128, TCW], F32, tag="t1", name="t1")
                    nc.vector.tensor_mul(t1[:], qf[:], cos_t[:])
                    t2 = scr.tile([128, TCW], F32, tag="t2", name="t2")
                    nc.vector.tensor_mul(t2[:], sw[:], sin_t[:])
                    nc.vector.tensor_add(t1[:], t1[:], t2[:])
                    dst = (qT[:, et * S + tt * TCW: et * S + tt * TCW + TCW]
                           if is_q else
                           kT[:, tt * TCW: tt * TCW + TCW])
                    nc.vector.tensor_mul(dst, t1[:], invb[:])

                def process_v(ps, tt):
                    vb = smol.tile([128, TCW], BF16, tag="vb", name="vb")
                    nc.scalar.copy(vb[:], ps[:])
                    for bb in range(TCW // 128):
                        tp = pvt.tile([128, 128], BF16, tag="tp", name="tp")
                        nc.tensor.transpose(
                            tp[:], vb[:, bb * 128:(bb + 1) * 128], idn[:])
                        blk = tt * (TCW // 128) + bb
                        nc.scalar.copy(V[:, blk * 128:(blk + 1) * 128], tp[:])

                pending = None  # (psum, et, tt, cos_t, sin_t)
                for tt in range(NTT):
                    xt = xpool.tile([128, NDT * TCW], BF16, tag="x")
                    for dt in range(NDT):
                        if tt == 0:  # interleave weight panels in need-order
                            nc.sync.dma_start(
                                w_sb[:, dt * EW:(dt + 1) * EW],
                                w_d[dt * 128:(dt + 1) * 128, :])
                        nc.scalar.dma_start(
                            xt[:, dt * TCW:(dt + 1) * TCW],
                            xT_d[dt * 128:(dt + 1) * 128,
                                 tt * TCW:(tt + 1) * TCW])
                    cos_t = cspool.tile([128, TCW], F32, tag="cos")
                    nc.sync.dma_start(cos_t[:], cos_d[:, tt * TCW:(tt + 1) * TCW])
                    sin_t = cspool.tile([128, TCW], F32, tag="sin")
                    nc.sync.dma_start(sin_t[:], sin_d[:, tt * TCW:(tt + 1) * TCW])

                    for et in range(HL + 2):
                        ps = p1.tile([128, TCW], F32, tag="ps")
                        for dt in range(NDT):
                            nc.tensor.matmul(
                                ps[:],
                                w_sb[:, dt * EW + et * 128:dt * EW + (et + 1) * 128],
                                xt[:, dt * TCW:(dt + 1) * TCW],
                                start=(dt == 0), stop=(dt == NDT - 1),
                            )
                        # process the PREVIOUS tile now: its cross-engine
                        # waits overlap this tile's matmul group
                        if pending is not None:
                            pps, pet, ptt, pc, psn_ = pending
                            if pet < HL + 1:
                                process_qk(pps, pet, ptt, pc, psn_)
                            else:
                                process_v(pps, ptt)
                        pending = (ps, et, tt, cos_t, sin_t)
                pps, pet, ptt, pc, psn_ = pending
                if pet < HL + 1:
                    process_qk(pps, pet, ptt, pc, psn_)
                else:
                    process_v(pps, ptt)

            # ---------------- Phase 2: causal attention + outproj ----------------
            with tc.tile_pool(name="wo", bufs=1) as wopool, \
                 tc.tile_pool(name="part", bufs=1) as partpool, \
                 tc.tile_pool(name="yf", bufs=1) as yfpool, \
                 tc.tile_pool(name="yt", bufs=2) as ytpool, \
                 tc.tile_pool(name="acc", bufs=2) as accpool, \
                 tc.tile_pool(name="exp", bufs=8) as epool, \
                 tc.tile_pool(name="rs", bufs=2) as rspool, \
                 tc.tile_pool(name="ot", bufs=2) as otpool, \
                 tc.tile_pool(name="pa", bufs=2, space="PSUM") as pa, \
                 tc.tile_pool(name="py", bufs=2, space="PSUM") as py, \
                 tc.tile_pool(name="po", bufs=2, space="PSUM") as po:

                part = partpool.tile([128, NDT * TPT], F32, tag="part")
                wo_h = [wopool.tile([128, 4 * S], BF16, tag=f"wo{h}",
                                    name=f"wo{h}")
                        for h in range(HL)]
                yf_h = [yfpool.tile([128, 4 * TPT], BF16, tag=f"yf{h}",
                                    name=f"yf{h}")
                        for h in range(HL)]
                ag_out = []  # (out_b, h) awaiting readback
                pid = nc.gpsimd.partition_id()
                roff = (pid % 4) * TPT

                def readback(h):
                    out_b = ag_out[h]
                    for r in range(4):
                        nc.gpsimd.dma_start(
                            yf_h[h][:, r * TPT:(r + 1) * TPT],
                            out_b[r * 128:(r + 1) * 128, bass.ds(roff, TPT)])

                for h in range(HL):
                    # prefetch this head's output-projection weight panels
                    for j in range(4):
                        et = 4 * j + h
                        nc.sync.dma_start(
                            wo_h[h][:, j * S:(j + 1) * S],
                            wo_d[et * 128:(et + 1) * 128, :])
                    yT = ytpool.tile([128, S], BF16, tag="yT", name="yT")
                    for qc in range(NQC):
                        nblk = 4 * (qc + 1)
                        nfull = 4 * qc
                        npair = nblk // 2
                        ps_y = py.tile([128, QC], F32, tag="y", name="ps_y")
                        acc = accpool.tile([128, QC], F32, tag="acc", name="acc")
                        qsl = qT[:, h * S + qc * QC: h * S + (qc + 1) * QC]

                        pend_av = []  # (ex2, ga) pairs awaiting AV matmuls

                        def emit_av(ex2, ga):
                            for g, off in ((ga, 0), (ga + 1, QC)):
                                w0 = max(0, g - nfull) * KB
                                nc.tensor.matmul(
                                    ps_y[:, w0:QC],
                                    V[:, g * 128:(g + 1) * 128],
                                    ex2[:, off + w0: off + QC],
                                    start=(g == 0), stop=(g == nblk - 1))

                        for p in range(npair):
                            ga = 2 * p
                            pa2 = pa.tile([128, 2 * QC], F32, tag="s", name="pa2")
                            nc.tensor.matmul(
                                pa2[:, 0:QC],
                                kT[:, ga * KB:(ga + 1) * KB],
                                qsl, start=True, stop=True)
                            nc.tensor.matmul(
                                pa2[:, QC:2 * QC],
                                kT[:, (ga + 1) * KB:(ga + 2) * KB],
                                qsl, start=True, stop=True)
                            ex2 = epool.tile([128, 2 * QC], BF16, tag="e",
                                             name="ex2")
                            nc.scalar.activation(
                                ex2[:], pa2[:],
                                mybir.ActivationFunctionType.Exp, bias=b0[:])
                            if ga >= nfull:  # diagonal pair: causal mask
                                ta = ga - nfull
                                nc.vector.tensor_mul(
                                    ex2[:], ex2[:],
                                    msk[:, ta * QC:(ta + 2) * QC])
                            # denominator accumulation on gpsimd
                            if p == 0:
                                nc.gpsimd.tensor_add(
                                    acc[:], ex2[:, 0:QC], ex2[:, QC:2 * QC])
                            else:
                                nc.gpsimd.tensor_add(
                                    acc[:], acc[:], ex2[:, 0:QC])
                                nc.gpsimd.tensor_add(
                                    acc[:], acc[:], ex2[:, QC:2 * QC])
                            pend_av.append((ex2, ga))
                            if len(pend_av) > AVDEPTH:
                                emit_av(*pend_av.pop(0))
                        for args in pend_av:
                            emit_av(*args)

                        # late readback of an earlier head's gather (keeps the
                        # gpsimd queue clear of long collective waits)
                        if qc == 0 and h >= 2:
                            readback(h - 2)

                        denb = rspool.tile([128, QC], F32, tag="den",
                                           name="denb")
                        nc.gpsimd.partition_all_reduce(
                            denb[:], acc[:], channels=128,
                            reduce_op=bass_isa.ReduceOp.add)
                        rec = rspool.tile([128, QC], F32, tag="rec", name="rec")
                        nc.vector.reciprocal_approx_fast(rec[:], denb[:])
                        nc.vector.tensor_mul(
                            yT[:, qc * QC:(qc + 1) * QC], ps_y[:], rec[:])

                    # per-head AllGather of y, overlapped with later heads
                    in_b = dpool.tile([128, S], BF16, tag=f"agin{h}",
                                      name=f"agin{h}")
                    out_b = dpool.tile([4 * 128, S], BF16, tag=f"agout{h}",
                                       name=f"agout{h}")
                    nc.sync.dma_start(in_b[:], yT[:])
                    nc.gpsimd.collective_compute(
                        "AllGather", mybir.AluOpType.bypass,
                        replica_groups=RG,
                        ins=[in_b.opt()], outs=[out_b.opt()])
                    ag_out.append(out_b)

                readback(HL - 2)
                readback(HL - 1)

                # ---- output projection: one pass per head, accumulated ----
                for h in range(HL):
                    last = (h == HL - 1)
                    for ot in range(NDT):
                        ps_o = po.tile([128, TPT], F32, tag="o", name="ps_o")
                        for j in range(4):
                            nc.tensor.matmul(
                                ps_o[:],
                                wo_h[h][:, j * S + ot * 128: j * S + ot * 128 + 128],
                                yf_h[h][:, j * TPT:(j + 1) * TPT],
                                start=(j == 0), stop=(j == 3))
                        psl = part[:, ot * TPT:(ot + 1) * TPT]
                        if h == 0:
                            nc.vector.tensor_copy(psl, ps_o[:])
                        elif not last:
                            nc.vector.tensor_add(psl, psl, ps_o[:])
                        else:
                            ott = otpool.tile([128, TPT], BF16, tag="ot",
                                              name="ott")
                            nc.vector.tensor_add(ott[:], ps_o[:], psl)
                            nc.sync.dma_start(
                                out_d[ot * 128:(ot + 1) * 128, :], ott[:])

    nc.compile()
    return nc


def make_in_maps(x, freqs_cis, wqkv, wo, q_norm_w, k_norm_w, S):
    """Host-side sharding / layout prep. Returns list of 8 input dicts."""
    bf = ml_dtypes.bfloat16
    QC = 512
    KB = 128

    # rope tables: [128, S]; row 2i & 2i+1 carry cos[t, i]; sin signed
    cos = np.asarray(freqs_cis[:S, :, 0], np.float32)   # [S, 64]
    sin = np.asarray(freqs_cis[:S, :, 1], np.float32)
    cosF = np.repeat(cos.T, 2, axis=0).astype(np.float32)      # [128, S]
    sinF = np.repeat(sin.T, 2, axis=0).astype(np.float32)
    sinF[0::2] *= -1.0
    cosF = np.ascontiguousarray(cosF)
    sinF = np.ascontiguousarray(sinF)

    swapP = np.zeros((128, 128), np.float32)
    for i in range(64):
        swapP[2 * i, 2 * i + 1] = 1.0
        swapP[2 * i + 1, 2 * i] = 1.0
    swapP = swapP.astype(bf)
    ident = np.eye(128, dtype=bf)

    # masks [KB, (QC//KB)*QC]: pattern t for the t-th kv block inside the
    # diagonal QC-region: allowed iff (t*KB + r) <= c
    r = np.arange(KB)[:, None]
    c = np.arange(QC)[None, :]
    pats = [((t * KB + r) <= c).astype(np.float32) for t in range(QC // KB)]
    masks = np.concatenate(pats, axis=1).astype(bf)

    qwv = np.asarray(q_norm_w, np.float32).reshape(128, 1)
    kwv = np.asarray(k_norm_w, np.float32).reshape(128, 1)

    woT = np.ascontiguousarray(np.asarray(wo, np.float32).T).astype(bf)

    xTb = []
    for b in range(2):
        xTb.append(np.ascontiguousarray(np.asarray(x[b], np.float32).T)
                   .astype(bf))

    wq = np.asarray(wqkv, np.float32)
    q_sz = NH * HD
    in_maps = []
    for c_id in range(N_CORES):
        b, rk = c_id // 4, c_id % 4
        rows = np.concatenate([
            wq[rk * HL * HD:(rk + 1) * HL * HD],          # 4 q heads
            wq[q_sz + rk * HD: q_sz + (rk + 1) * HD],     # k head
            wq[q_sz + NKV * HD + rk * HD:
               q_sz + NKV * HD + (rk + 1) * HD],          # v head
        ], axis=0)                                        # [768, 2048]
        wslice = np.ascontiguousarray(rows.T).astype(bf)  # [2048, 768]
        in_maps.append({
            "xT": xTb[b], "wslice": wslice, "woT": woT,
            "cosF": cosF, "sinF": sinF, "swapP": swapP,
            "ident": ident, "masks": masks,
            "qw": qwv, "kw": kwv,
        })
    return in_maps


_NC_CACHE = {}


def kernel(x, freqs_cis, mask, wqkv, wo, q_norm_w, k_norm_w):
    x = np.asarray(x)
    S = x.shape[1]
    if S not in _NC_CACHE:
        _NC_CACHE[S] = build_graph(S)
    nc = _NC_CACHE[S]
    in_maps = make_in_maps(x, freqs_cis, wqkv, wo, q_norm_w, k_norm_w, S)
    res = run_bass_kernel_spmd(nc, in_maps, core_ids=list(range(N_CORES)))
    TPT = S // 4
    out = np.empty((2, S, DIM), np.float32)
    for c_id in range(N_CORES):
        b, rk = c_id // 4, c_id % 4
        out[b, rk * TPT:(rk + 1) * TPT, :] = res.results[c_id]["out"].T.astype(np.float32)
    return out


# revision 13
# speedup vs baseline: 1.5970x; 1.5970x over previous
"""Distributed Trainium2 kernel for nn_Attention (dense transformer block:
fused QKV projection + per-head RMSNorm + rotary + causal GQA attention + output
projection), running SPMD on 8 NeuronCores.

Sharding (rank-uniform, no divergent control flow):
  - 8 cores = 2 batch groups x 4 tensor-parallel ranks.
  - Core c: batch b = c // 4, rank r = c % 4.
  - QKV projection + attention are head-sharded: core r computes q heads
    4r..4r+3 and kv head r for ALL tokens of its batch.
  - Per-head AllGather re-shards y from head-split to token-split, overlapped
    with the next head's attention; the output projection then runs locally
    with the full contraction dim in 4 per-head passes (no all-reduce).

Layout tricks:
  - Host pre-transposes x, wqkv, wo so the kernel's matmuls need no on-device
    transposes (except tiny 128x128 PE transposes for V).
  - Rope's even/odd pair swap is a 128x128 permutation matmul in bf16
    (cheap; fp32 matmuls run at quarter rate).
  - Scores are computed transposed [kv, q]; exp is fused into the PSUM->SBUF
    eviction on the ScalarEngine, batched two kv-blocks per activation to
    amortize the ~300-cycle instruction overhead.
  - The softmax denominator is accumulated on the GpSimd engine (tensor_add
    over the exp tiles) and reduced across partitions with
    partition_all_reduce -- zero TensorEngine cost.
  - RMSNorm's sum-of-squares also uses partition_all_reduce instead of a
    ones-vector matmul; the 1/sqrt(head_dim) score scale folds into the
    q-side scalar.
  - All big matmuls run in bf16 with f32 PSUM accumulation.
"""

import numpy as np
import ml_dtypes

import concourse.bass as bass
import concourse.bass_isa as bass_isa
import concourse.mybir as mybir
import concourse.tile as tile
from concourse import bacc
from concourse.bass_utils import run_bass_kernel_spmd

BF16 = mybir.dt.bfloat16
F32 = mybir.dt.float32

DIM = 2048
NH = 16
NKV = 4
HD = 128
EPS = 1e-5
N_CORES = 8
RG = [[0, 1, 2, 3], [4, 5, 6, 7]]  # per-batch tensor-parallel groups

HL = NH // NKV  # q heads per core (= GQA group size) = 4
EW = HL * HD + 2 * HD  # wqkv column-slice width per core = 768
NDT = DIM // 128  # contraction tiles = 16


def build_graph(S):
    """Build + compile the SPMD graph for sequence length S. Returns nc."""
    TPT = S // 4       # tokens per core after the gather (output rows per core)
    TCW = S // 4       # token chunk width for phase 1 (moving dim <= 512)
    NTT = S // TCW     # number of token chunks = 4
    QC = 512           # attention q-chunk width
    KB = 128           # kv block size
    NQC = S // QC      # q chunks per head
    NB = S // 128      # 128-token blocks (for V layout)
    AVDEPTH = 3        # kv-block pairs the AV matmuls trail the score matmuls

    nc = bacc.Bacc("TRN2", target_bir_lowering=False, debug=False,
                   num_devices=N_CORES)

    # ---- DRAM I/O ----
    xT_d = nc.dram_tensor("xT", [DIM, S], BF16, kind="ExternalInput")
    w_d = nc.dram_tensor("wslice", [DIM, EW], BF16, kind="ExternalInput")
    wo_d = nc.dram_tensor("woT", [DIM, DIM], BF16, kind="ExternalInput")
    cos_d = nc.dram_tensor("cosF", [128, S], F32, kind="ExternalInput")
    sin_d = nc.dram_tensor("sinF", [128, S], F32, kind="ExternalInput")
    swp_d = nc.dram_tensor("swapP", [128, 128], BF16, kind="ExternalInput")
    idn_d = nc.dram_tensor("ident", [128, 128], BF16, kind="ExternalInput")
    msk_d = nc.dram_tensor("masks", [KB, (QC // KB) * QC], BF16, kind="ExternalInput")
    qw_d = nc.dram_tensor("qw", [128, 1], F32, kind="ExternalInput")
    kw_d = nc.dram_tensor("kw", [128, 1], F32, kind="ExternalInput")
    out_d = nc.dram_tensor("out", [DIM, TPT], BF16, kind="ExternalOutput")

    with tile.TileContext(nc) as tc:
        with tc.tile_pool(name="const", bufs=1) as cpool, \
             tc.tile_pool(name="big", bufs=1) as bigpool, \
             tc.tile_pool(name="dram", bufs=1, space="DRAM") as dpool:

            # constants
            swp = cpool.tile([128, 128], BF16, tag="swp")
            nc.sync.dma_start(swp[:], swp_d[:])
            idn = cpool.tile([128, 128], BF16, tag="idn")
            nc.sync.dma_start(idn[:], idn_d[:])
            msk = cpool.tile([KB, (QC // KB) * QC], BF16, tag="msk")
            nc.sync.dma_start(msk[:], msk_d[:])
            qw = cpool.tile([128, 1], F32, tag="qw")
            nc.sync.dma_start(qw[:], qw_d[:])
            kw = cpool.tile([128, 1], F32, tag="kw")
            nc.sync.dma_start(kw[:], kw_d[:])
            b0 = cpool.tile([128, 1], F32, tag="b0")
            nc.vector.memset(b0[:], 0.0)
            bq = cpool.tile([1, 1], F32, tag="bq")
            nc.vector.memset(bq[:], float(HD * EPS))
            bk = cpool.tile([1, 1], F32, tag="bk")
            nc.vector.memset(bk[:], float(EPS))

            # long-lived activations
            qT = bigpool.tile([128, HL * S], BF16, tag="qT")
            kT = bigpool.tile([128, S], BF16, tag="kT")
            V = bigpool.tile([128, S], BF16, tag="V")   # [tok%128, blk*128+d]

            # ---------------- Phase 1: QKV + norm + rope ----------------
            with tc.tile_pool(name="wq", bufs=1) as wpool, \
                 tc.tile_pool(name="x", bufs=2) as xpool, \
                 tc.tile_pool(name="cs", bufs=2) as cspool, \
                 tc.tile_pool(name="scr", bufs=2) as scr, \
                 tc.tile_pool(name="smol", bufs=2) as smol, \
                 tc.tile_pool(name="p1", bufs=3, space="PSUM") as p1, \
                 tc.tile_pool(name="psw", bufs=2, space="PSUM") as psw, \
                 tc.tile_pool(name="pvt", bufs=2, space="PSUM") as pvt:

                # full wqkv slice, staged once: [128, dt*EW + e]
                w_sb = wpool.tile([128, NDT * EW], BF16, tag="w")

                def process_qk(ps, et, tt, cos_t, sin_t):
                    is_q = et < HL
                    # sum of squares over head_dim via gpsimd partition reduce
                    sqv = smol.tile([128, TCW], F32, tag="sq2", name="sqv")
                    nc.scalar.activation(
                        sqv[:], ps[:],
                        mybir.ActivationFunctionType.Square, bias=b0[:])
                    ssb = scr.tile([128, TCW], F32, tag="ssb", name="ssb")
                    nc.gpsimd.partition_all_reduce(
                        ssb[:], sqv[:], channels=128,
                        reduce_op=bass_isa.ReduceOp.add)
                    sq = smol.tile([1, TCW], F32, tag="sqs", name="sq")
                    if is_q:
                        # 1/sqrt(ss + HD*eps) folds the 1/sqrt(HD) score scale
                        nc.scalar.activation(
                            sq[:], ssb[0:1, :],
                            mybir.ActivationFunctionType.Sqrt,
                            bias=bq[:], scale=1.0)
                    else:
                        nc.scalar.activation(
                            sq[:], ssb[0:1, :],
                            mybir.ActivationFunctionType.Sqrt,
                            bias=bk[:], scale=1.0 / HD)
                    inv = smol.tile([1, TCW], F32, tag="inv", name="inv")
                    nc.vector.reciprocal_approx_fast(inv[:], sq[:])
                    invb = scr.tile([128, TCW], F32, tag="invb", name="invb")
                    nc.gpsimd.partition_broadcast(invb[:], inv[:])
                    qf = scr.tile([128, TCW], BF16, tag="qf", name="qf")
                    nc.scalar.mul(qf[:], ps[:], (qw if is_q else kw)[:])
                    # rope: pair swap via bf16 permutation matmul, sinF signed
                    sw = psw.tile([128, TCW], F32, tag="sw", name="sw")
                    nc.tensor.matmul(sw[:], swp[:], qf[:],
                                     start=True, stop=True)
                    t1 = scr.tile([128, TCW], F32, tag="t1", name="t1")
                    nc.vector.tensor_mul(t1[:], qf[:], cos_t[:])
                    t2 = scr.tile([128, TCW], F32, tag="t2", name="t2")
                    nc.vector.tensor_mul(t2[:], sw[:], sin_t[:])
                    nc.vector.tensor_add(t1[:], t1[:], t2[:])
                    dst = (qT[:, et * S + tt * TCW: et * S + tt * TCW + TCW]
                           if is_q else
                           kT[:, tt * TCW: tt * TCW + TCW])
                    nc.vector.tensor_mul(dst, t1[:], invb[:])

                def process_v(ps, tt):
                    vb = smol.tile([128, TCW], BF16, tag="vb", name="vb")
                    nc.scalar.copy(vb[:], ps[:])
                    for bb in range(TCW // 128):
                        tp = pvt.tile([128, 128], BF16, tag="tp", name="tp")
                        nc.tensor.transpose(
                            tp[:], vb[:, bb * 128:(bb + 1) * 128], idn[:])
                        blk = tt * (TCW // 128) + bb
                        nc.scalar.copy(V[:, blk * 128:(blk + 1) * 128], tp[:])

                pending = None  # (psum, et, tt, cos_t, sin_t)
                for tt in range(NTT):
                    xt = xpool.tile([128, NDT * TCW], BF16, tag="x")
                    for dt in range(NDT):
                        if tt == 0:  # interleave weight panels in need-order
                            nc.sync.dma_start(
                                w_sb[:, dt * EW:(dt + 1) * EW],
                                w_d[dt * 128:(dt + 1) * 128, :])
                        nc.scalar.dma_start(
                            xt[:, dt * TCW:(dt + 1) * TCW],
                            xT_d[dt * 128:(dt + 1) * 128,
                                 tt * TCW:(tt + 1) * TCW])
                    cos_t = cspool.tile([128, TCW], F32, tag="cos")
                    nc.sync.dma_start(cos_t[:], cos_d[:, tt * TCW:(tt + 1) * TCW])
                    sin_t = cspool.tile([128, TCW], F32, tag="sin")
                    nc.sync.dma_start(sin_t[:], sin_d[:, tt * TCW:(tt + 1) * TCW])

                    for et in range(HL + 2):
                        ps = p1.tile([128, TCW], F32, tag="ps")
                        for dt in range(NDT):
                            nc.tensor.matmul(
                                ps[:],
                                w_sb[:, dt * EW + et * 128:dt * EW + (et + 1) * 128],
                                xt[:, dt * TCW:(dt + 1) * TCW],
                                start=(dt == 0), stop=(dt == NDT - 1),
                            )
                        # process the PREVIOUS tile now: its cross-engine
                        # waits overlap this tile's matmul group
                        if pending is not None:
                            pps, pet, ptt, pc, psn_ = pending
                            if pet < HL + 1:
                                process_qk(pps, pet, ptt, pc, psn_)
                            else:
                                process_v(pps, ptt)
                        pending = (ps, et, tt, cos_t, sin_t)
                pps, pet, ptt, pc, psn_ = pending
                if pet < HL + 1:
                    process_qk(pps, pet, ptt, pc, psn_)
                else:
                    process_v(pps, ptt)

            # ---------------- Phase 2: causal attention + outproj ----------------
            with tc.tile_pool(name="wo", bufs=1) as wopool, \
                 tc.tile_pool(name="part", bufs=1) as partpool, \
                 tc.tile_pool(name="yf", bufs=1) as yfpool, \
                 tc.tile_pool(name="yt", bufs=2) as ytpool, \
                 tc.tile_pool(name="acc", bufs=2) as accpool, \
                 tc.tile_pool(name="exp", bufs=8) as epool, \
                 tc.tile_pool(name="rs", bufs=2) as rspool, \
                 tc.tile_pool(name="ot", bufs=2) as otpool, \
                 tc.tile_pool(name="pa", bufs=2, space="PSUM") as pa, \
                 tc.tile_pool(name="py", bufs=2, space="PSUM") as py, \
                 tc.tile_pool(name="po", bufs=2, space="PSUM") as po:

                part = partpool.tile([128, NDT * TPT], F32, tag="part")
                wo_h = [wopool.tile([128, 4 * S], BF16, tag=f"wo{h}",
                                    name=f"wo{h}")
                        for h in range(HL)]
                yf_h = [yfpool.tile([128, 4 * TPT], BF16, tag=f"yf{h}",
                                    name=f"yf{h}")
                        for h in range(HL)]
                ag_out = []  # (out_b, h) awaiting readback
                pid = nc.gpsimd.partition_id()
                roff = (pid % 4) * TPT

                def readback(h):
                    out_b = ag_out[h]
                    for r in range(4):
                        nc.gpsimd.dma_start(
                            yf_h[h][:, r * TPT:(r + 1) * TPT],
                            out_b[r * 128:(r + 1) * 128, bass.ds(roff, TPT)])

                for h in range(HL):
                    # prefetch this head's output-projection weight panels
                    for j in range(4):
                        et = 4 * j + h
                        nc.sync.dma_start(
                            wo_h[h][:, j * S:(j + 1) * S],
                            wo_d[et * 128:(et + 1) * 128, :])
                    yT = ytpool.tile([128, S], BF16, tag="yT", name="yT")
                    for qc in range(NQC):
                        nblk = 4 * (qc + 1)
                        nfull = 4 * qc
                        npair = nblk // 2
                        ps_y = py.tile([128, QC], F32, tag="y", name="ps_y")
                        acc = accpool.tile([128, QC], F32, tag="acc", name="acc")
                        qsl = qT[:, h * S + qc * QC: h * S + (qc + 1) * QC]

                        pend_av = []  # (ex2, ga) pairs awaiting AV matmuls

                        def emit_av(ex2, ga):
                            for g, off in ((ga, 0), (ga + 1, QC)):
                                w0 = max(0, g - nfull) * KB
                                nc.tensor.matmul(
                                    ps_y[:, w0:QC],
                                    V[:, g * 128:(g + 1) * 128],
                                    ex2[:, off + w0: off + QC],
                                    start=(g == 0), stop=(g == nblk - 1))

                        for p in range(npair):
                            ga = 2 * p
                            pa2 = pa.tile([128, 2 * QC], F32, tag="s", name="pa2")
                            nc.tensor.matmul(
                                pa2[:, 0:QC],
                                kT[:, ga * KB:(ga + 1) * KB],
                                qsl, start=True, stop=True)
                            nc.tensor.matmul(
                                pa2[:, QC:2 * QC],
                                kT[:, (ga + 1) * KB:(ga + 2) * KB],
                                qsl, start=True, stop=True)
                            ex2 = epool.tile([128, 2 * QC], BF16, tag="e",
                                             name="ex2")
                            nc.scalar.activation(
                                ex2[:], pa2[:],
                                mybir.ActivationFunctionType.Exp, bias=b0[:])
                            if ga >= nfull:  # diagonal pair: causal mask
                                ta = ga - nfull
                                nc.vector.tensor_mul(
                                    ex2[:], ex2[:],
                                    msk[:, ta * QC:(ta + 2) * QC])
                            # denominator accumulation on DVE: bf16 pair sum
                            # (one rounding) then exact f32 accumulate
                            ap = epool.tile([128, QC], BF16, tag="ap",
                                            name="accp")
                            nc.vector.tensor_add(
                                ap[:], ex2[:, 0:QC], ex2[:, QC:2 * QC])
                            if p == 0:
                                nc.vector.tensor_copy(acc[:], ap[:])
                            else:
                                nc.vector.tensor_add(acc[:], acc[:], ap[:])
                            pend_av.append((ex2, ga))
                            if len(pend_av) > AVDEPTH:
                                emit_av(*pend_av.pop(0))
                        for args in pend_av:
                            emit_av(*args)

                        # late readback of an earlier head's gather (keeps the
                        # gpsimd queue clear of long collective waits)
                        if qc == 0 and h >= 2:
                            readback(h - 2)

                        denb = rspool.tile([128, QC], F32, tag="den",
                                           name="denb")
                        nc.gpsimd.partition_all_reduce(
                            denb[:], acc[:], channels=128,
                            reduce_op=bass_isa.ReduceOp.add)
                        rec = rspool.tile([128, QC], F32, tag="rec", name="rec")
                        nc.vector.reciprocal_approx_fast(rec[:], denb[:])
                        nc.vector.tensor_mul(
                            yT[:, qc * QC:(qc + 1) * QC], ps_y[:], rec[:])

                    # per-head AllGather of y, overlapped with later heads
                    in_b = dpool.tile([128, S], BF16, tag=f"agin{h}",
                                      name=f"agin{h}")
                    out_b = dpool.tile([4 * 128, S], BF16, tag=f"agout{h}",
                                       name=f"agout{h}")
                    nc.sync.dma_start(in_b[:], yT[:])
                    nc.gpsimd.collective_compute(
                        "AllGather", mybir.AluOpType.bypass,
                        replica_groups=RG,
                        ins=[in_b.opt()], outs=[out_b.opt()])
                    ag_out.append(out_b)

                readback(HL - 2)
                readback(HL - 1)

                # ---- output projection: one pass per head, accumulated ----
                for h in range(HL):
                    last = (h == HL - 1)
                    for ot in range(NDT):
                        ps_o = po.tile([128, TPT], F32, tag="o", name="ps_o")
                        for j in range(4):
                            nc.tensor.matmul(
                                ps_o[:],
                                wo_h[h][:, j * S + ot * 128: j * S + ot * 128 + 128],
                                yf_h[h][:, j * TPT:(j + 1) * TPT],
                                start=(j == 0), stop=(j == 3))
                        psl = part[:, ot * TPT:(ot + 1) * TPT]
                        if h == 0:
                            nc.vector.tensor_copy(psl, ps_o[:])
                        elif not last:
                            nc.vector.tensor_add(psl, psl, ps_o[:])
                        else:
                            ott = otpool.tile([128, TPT], BF16, tag="ot",
                                              name="ott")
                            nc.vector.tensor_add(ott[:], ps_o[:], psl)
                            nc.sync.dma_start(
                                out_d[ot * 128:(ot + 1) * 128, :], ott[:])

    nc.compile()
    return nc


def make_in_maps(x, freqs_cis, wqkv, wo, q_norm_w, k_norm_w, S):
    """Host-side sharding / layout prep. Returns list of 8 input dicts."""
    bf = ml_dtypes.bfloat16
    QC = 512
    KB = 128

    # rope tables: [128, S]; row 2i & 2i+1 carry cos[t, i]; sin signed
    cos = np.asarray(freqs_cis[:S, :, 0], np.float32)   # [S, 64]
    sin = np.asarray(freqs_cis[:S, :, 1], np.float32)
    cosF = np.repeat(cos.T, 2, axis=0).astype(np.float32)      # [128, S]
    sinF = np.repeat(sin.T, 2, axis=0).astype(np.float32)
    sinF[0::2] *= -1.0
    cosF = np.ascontiguousarray(cosF)
    sinF = np.ascontiguousarray(sinF)

    swapP = np.zeros((128, 128), np.float32)
    for i in range(64):
        swapP[2 * i, 2 * i + 1] = 1.0
        swapP[2 * i + 1, 2 * i] = 1.0
    swapP = swapP.astype(bf)
    ident = np.eye(128, dtype=bf)

    # masks [KB, (QC//KB)*QC]: pattern t for the t-th kv block inside the
    # diagonal QC-region: allowed iff (t*KB + r) <= c
    r = np.arange(KB)[:, None]
    c = np.arange(QC)[None, :]
    pats = [((t * KB + r) <= c).astype(np.float32) for t in range(QC // KB)]
    masks = np.concatenate(pats, axis=1).astype(bf)

    qwv = np.asarray(q_norm_w, np.float32).reshape(128, 1)
    kwv = np.asarray(k_norm_w, np.float32).reshape(128, 1)

    woT = np.ascontiguousarray(np.asarray(wo, np.float32).T).astype(bf)

    xTb = []
    for b in range(2):
        xTb.append(np.ascontiguousarray(np.asarray(x[b], np.float32).T)
                   .astype(bf))

    wq = np.asarray(wqkv, np.float32)
    q_sz = NH * HD
    in_maps = []
    for c_id in range(N_CORES):
        b, rk = c_id // 4, c_id % 4
        rows = np.concatenate([
            wq[rk * HL * HD:(rk + 1) * HL * HD],          # 4 q heads
            wq[q_sz + rk * HD: q_sz + (rk + 1) * HD],     # k head
            wq[q_sz + NKV * HD + rk * HD:
               q_sz + NKV * HD + (rk + 1) * HD],          # v head
        ], axis=0)                                        # [768, 2048]
        wslice = np.ascontiguousarray(rows.T).astype(bf)  # [2048, 768]
        in_maps.append({
            "xT": xTb[b], "wslice": wslice, "woT": woT,
            "cosF": cosF, "sinF": sinF, "swapP": swapP,
            "ident": ident, "masks": masks,
            "qw": qwv, "kw": kwv,
        })
    return in_maps


_NC_CACHE = {}


def kernel(x, freqs_cis, mask, wqkv, wo, q_norm_w, k_norm_w):
    x = np.asarray(x)
    S = x.shape[1]
    if S not in _NC_CACHE:
        _NC_CACHE[S] = build_graph(S)
    nc = _NC_CACHE[S]
    in_maps = make_in_maps(x, freqs_cis, wqkv, wo, q_norm_w, k_norm_w, S)
    res = run_bass_kernel_spmd(nc, in_maps, core_ids=list(range(N_CORES)))
    TPT = S // 4
    out = np.empty((2, S, DIM), np.float32)
    for c_id in range(N_CORES):
        b, rk = c_id // 4, c_id % 4
        out[b, rk * TPT:(rk + 1) * TPT, :] = res.results[c_id]["out"].T.astype(np.float32)
    return out


# revision 25
# speedup vs baseline: 1.7377x; 1.0881x over previous
"""Distributed Trainium2 kernel for nn_Attention (dense transformer block:
fused QKV projection + per-head RMSNorm + rotary + causal GQA attention + output
projection), running SPMD on 8 NeuronCores.

Sharding (rank-uniform, no divergent control flow):
  - 8 cores = 2 batch groups x 4 tensor-parallel ranks.
  - Core c: batch b = c // 4, rank r = c % 4.
  - QKV projection + attention are head-sharded: core r computes q heads
    4r..4r+3 and kv head r for ALL tokens of its batch.
  - Per-head AllGather re-shards y from head-split to token-split, overlapped
    with the next head's attention; the output projection then runs locally
    with the full contraction dim in 4 per-head passes (no all-reduce).

Layout tricks:
  - Host pre-transposes x, wqkv, wo so the kernel's matmuls need no on-device
    transposes (except tiny 128x128 PE transposes for V).
  - Rope's even/odd pair swap is a 128x128 permutation matmul in bf16
    (cheap; fp32 matmuls run at quarter rate).
  - Scores are computed transposed [kv, q]; exp is fused into the PSUM->SBUF
    eviction on the ScalarEngine, batched two kv-blocks per activation to
    amortize the ~300-cycle instruction overhead.
  - The softmax denominator is accumulated on the GpSimd engine (tensor_add
    over the exp tiles) and reduced across partitions with
    partition_all_reduce -- zero TensorEngine cost.
  - RMSNorm's sum-of-squares also uses partition_all_reduce instead of a
    ones-vector matmul; the 1/sqrt(head_dim) score scale folds into the
    q-side scalar.
  - All big matmuls run in bf16 with f32 PSUM accumulation.
"""

import numpy as np
import ml_dtypes

import concourse.bass as bass
import concourse.bass_isa as bass_isa
import concourse.mybir as mybir
import concourse.tile as tile
from concourse import bacc
from concourse.bass_utils import run_bass_kernel_spmd

BF16 = mybir.dt.bfloat16
F32 = mybir.dt.float32

DIM = 2048
NH = 16
NKV = 4
HD = 128
EPS = 1e-5
N_CORES = 8
RG = [[0, 1, 2, 3], [4, 5, 6, 7]]  # per-batch tensor-parallel groups

HL = NH // NKV  # q heads per core (= GQA group size) = 4
EW = HL * HD + 2 * HD  # wqkv column-slice width per core = 768
NDT = DIM // 128  # contraction tiles = 16


def build_graph(S):
    """Build + compile the SPMD graph for sequence length S. Returns nc."""
    TPT = S // 4       # tokens per core after the gather (output rows per core)
    TCW = S // 4       # token chunk width for phase 1 (moving dim <= 512)
    NTT = S // TCW     # number of token chunks = 4
    QC = 512           # attention q-chunk width
    KB = 128           # kv block size
    NQC = S // QC      # q chunks per head
    NB = S // 128      # 128-token blocks (for V layout)
    AVDEPTH = 3        # kv-block pairs the AV matmuls trail the score matmuls

    nc = bacc.Bacc("TRN2", target_bir_lowering=False, debug=False,
                   num_devices=N_CORES)

    # ---- DRAM I/O ----
    xT_d = nc.dram_tensor("xT", [DIM, S], BF16, kind="ExternalInput")
    w_d = nc.dram_tensor("wslice", [DIM, EW], BF16, kind="ExternalInput")
    wo_d = nc.dram_tensor("woT", [DIM, DIM], BF16, kind="ExternalInput")
    cos_d = nc.dram_tensor("cosF", [128, S], F32, kind="ExternalInput")
    sin_d = nc.dram_tensor("sinF", [128, S], F32, kind="ExternalInput")
    swp_d = nc.dram_tensor("swapP", [128, 128], BF16, kind="ExternalInput")
    idn_d = nc.dram_tensor("ident", [128, 128], BF16, kind="ExternalInput")
    msk_d = nc.dram_tensor("masks", [KB, (QC // KB) * QC], BF16, kind="ExternalInput")
    qw_d = nc.dram_tensor("qw", [128, 1], F32, kind="ExternalInput")
    kw_d = nc.dram_tensor("kw", [128, 1], F32, kind="ExternalInput")
    out_d = nc.dram_tensor("out", [DIM, TPT], BF16, kind="ExternalOutput")

    with tile.TileContext(nc) as tc:
        with tc.tile_pool(name="const", bufs=1) as cpool, \
             tc.tile_pool(name="big", bufs=1) as bigpool, \
             tc.tile_pool(name="dram", bufs=1, space="DRAM") as dpool:

            # constants
            swp = cpool.tile([128, 128], BF16, tag="swp")
            nc.sync.dma_start(swp[:], swp_d[:])
            idn = cpool.tile([128, 128], BF16, tag="idn")
            nc.sync.dma_start(idn[:], idn_d[:])
            msk = cpool.tile([KB, (QC // KB) * QC], BF16, tag="msk")
            nc.sync.dma_start(msk[:], msk_d[:])
            qw = cpool.tile([128, 1], F32, tag="qw")
            nc.sync.dma_start(qw[:], qw_d[:])
            kw = cpool.tile([128, 1], F32, tag="kw")
            nc.sync.dma_start(kw[:], kw_d[:])
            ones = cpool.tile([128, 1], BF16, tag="ones")
            nc.vector.memset(ones[:], 1.0)
            b0 = cpool.tile([128, 1], F32, tag="b0")
            nc.vector.memset(b0[:], 0.0)
            bq = cpool.tile([1, 1], F32, tag="bq")
            nc.vector.memset(bq[:], float(HD * EPS))
            bk = cpool.tile([1, 1], F32, tag="bk")
            nc.vector.memset(bk[:], float(EPS))

            # long-lived activations
            qT = bigpool.tile([128, HL * S], BF16, tag="qT")
            kT = bigpool.tile([128, S], BF16, tag="kT")
            V = bigpool.tile([128, S], BF16, tag="V")   # [tok%128, blk*128+d]

            # ---------------- Phase 1: QKV + norm + rope ----------------
            with tc.tile_pool(name="wq", bufs=1) as wpool, \
                 tc.tile_pool(name="x", bufs=1) as xpool, \
                 tc.tile_pool(name="cs", bufs=2) as cspool, \
                 tc.tile_pool(name="scr", bufs=2) as scr, \
                 tc.tile_pool(name="smol", bufs=2) as smol, \
                 tc.tile_pool(name="p1", bufs=3, space="PSUM") as p1, \
                 tc.tile_pool(name="psw", bufs=2, space="PSUM") as psw, \
                 tc.tile_pool(name="pss", bufs=1, space="PSUM") as pss, \
                 tc.tile_pool(name="pvt", bufs=2, space="PSUM") as pvt:

                # full wqkv slice, staged once: [128, dt*EW + e]
                w_sb = wpool.tile([128, NDT * EW], BF16, tag="w")
                # full x, staged once with large contiguous DMAs
                xfull = xpool.tile([128, NDT * S], BF16, tag="x")

                def process_qk(ps, et, tt, cos_t, sin_t):
                    is_q = et < HL
                    # sum of squares over head_dim via ones-vector matmul
                    sqv = smol.tile([128, TCW], BF16, tag="sq2", name="sqv")
                    nc.scalar.activation(
                        sqv[:], ps[:],
                        mybir.ActivationFunctionType.Square, bias=b0[:])
                    ss = pss.tile([1, TCW], F32, tag="ss", name="ss")
                    nc.tensor.matmul(ss[:], ones[:], sqv[:],
                                     start=True, stop=True)
                    sq = smol.tile([1, TCW], F32, tag="sqs", name="sq")
                    if is_q:
                        # 1/sqrt(ss + HD*eps) folds the 1/sqrt(HD) score scale
                        nc.scalar.activation(
                            sq[:], ss[:],
                            mybir.ActivationFunctionType.Sqrt,
                            bias=bq[:], scale=1.0)
                    else:
                        nc.scalar.activation(
                            sq[:], ss[:],
                            mybir.ActivationFunctionType.Sqrt,
                            bias=bk[:], scale=1.0 / HD)
                    inv = smol.tile([1, TCW], F32, tag="inv", name="inv")
                    nc.vector.reciprocal_approx_fast(inv[:], sq[:])
                    invb = scr.tile([128, TCW], F32, tag="invb", name="invb")
                    nc.gpsimd.partition_broadcast(invb[:], inv[:])
                    qf = scr.tile([128, TCW], BF16, tag="qf", name="qf")
                    nc.scalar.mul(qf[:], ps[:], (qw if is_q else kw)[:])
                    # rope: pair swap via bf16 permutation matmul, sinF signed
                    sw = psw.tile([128, TCW], F32, tag="sw", name="sw")
                    nc.tensor.matmul(sw[:], swp[:], qf[:],
                                     start=True, stop=True)
                    t1 = scr.tile([128, TCW], F32, tag="t1", name="t1")
                    nc.vector.tensor_mul(t1[:], qf[:], cos_t[:])
                    t2 = scr.tile([128, TCW], F32, tag="t2", name="t2")
                    nc.vector.tensor_mul(t2[:], sw[:], sin_t[:])
                    nc.vector.tensor_add(t1[:], t1[:], t2[:])
                    dst = (qT[:, et * S + tt * TCW: et * S + tt * TCW + TCW]
                           if is_q else
                           kT[:, tt * TCW: tt * TCW + TCW])
                    nc.vector.tensor_mul(dst, t1[:], invb[:])

                def process_v(ps, tt):
                    vb = smol.tile([128, TCW], BF16, tag="vb", name="vb")
                    nc.scalar.copy(vb[:], ps[:])
                    for bb in range(TCW // 128):
                        tp = pvt.tile([128, 128], BF16, tag="tp", name="tp")
                        nc.tensor.transpose(
                            tp[:], vb[:, bb * 128:(bb + 1) * 128], idn[:])
                        blk = tt * (TCW // 128) + bb
                        nc.scalar.copy(V[:, blk * 128:(blk + 1) * 128], tp[:])

                pending = None  # (psum, et, tt, cos_t, sin_t)
                for dt in range(NDT):  # first token chunk + weights, need-order
                    nc.sync.dma_start(
                        w_sb[:, dt * EW:(dt + 1) * EW],
                        w_d[dt * 128:(dt + 1) * 128, :])
                    nc.scalar.dma_start(
                        xfull[:, dt * S:dt * S + TCW],
                        xT_d[dt * 128:(dt + 1) * 128, 0:TCW])
                for dt in range(NDT):  # bulk of x with large contiguous DMAs
                    nc.scalar.dma_start(
                        xfull[:, dt * S + TCW:(dt + 1) * S],
                        xT_d[dt * 128:(dt + 1) * 128, TCW:])
                for tt in range(NTT):
                    cos_t = cspool.tile([128, TCW], F32, tag="cos")
                    nc.sync.dma_start(cos_t[:], cos_d[:, tt * TCW:(tt + 1) * TCW])
                    sin_t = cspool.tile([128, TCW], F32, tag="sin")
                    nc.sync.dma_start(sin_t[:], sin_d[:, tt * TCW:(tt + 1) * TCW])

                    for et in range(HL + 2):
                        ps = p1.tile([128, TCW], F32, tag="ps")
                        for dt in range(NDT):
                            nc.tensor.matmul(
                                ps[:],
                                w_sb[:, dt * EW + et * 128:dt * EW + (et + 1) * 128],
                                xfull[:, dt * S + tt * TCW:
                                      dt * S + (tt + 1) * TCW],
                                start=(dt == 0), stop=(dt == NDT - 1),
                            )
                        # process the PREVIOUS tile now: its cross-engine
                        # waits overlap this tile's matmul group
                        if pending is not None:
                            pps, pet, ptt, pc, psn_ = pending
                            if pet < HL + 1:
                                process_qk(pps, pet, ptt, pc, psn_)
                            else:
                                process_v(pps, ptt)
                        pending = (ps, et, tt, cos_t, sin_t)
                pps, pet, ptt, pc, psn_ = pending
                if pet < HL + 1:
                    process_qk(pps, pet, ptt, pc, psn_)
                else:
                    process_v(pps, ptt)

            # ---------------- Phase 2: causal attention + outproj ----------------
            with tc.tile_pool(name="wo", bufs=1) as wopool, \
                 tc.tile_pool(name="part", bufs=1) as partpool, \
                 tc.tile_pool(name="yf", bufs=1) as yfpool, \
                 tc.tile_pool(name="yt", bufs=2) as ytpool, \
                 tc.tile_pool(name="acc", bufs=2) as accpool, \
                 tc.tile_pool(name="exp", bufs=8) as epool, \
                 tc.tile_pool(name="rs", bufs=2) as rspool, \
                 tc.tile_pool(name="ot", bufs=2) as otpool:

                part = partpool.tile([128, NDT * TPT], F32, tag="part")
                wo_h = [wopool.tile([128, 4 * S], BF16, tag=f"wo{h}",
                                    name=f"wo{h}")
                        for h in range(HL)]
                yf_h = [yfpool.tile([128, 4 * TPT], BF16, tag=f"yf{h}",
                                    name=f"yf{h}")
                        for h in range(HL)]
                ag_out = []  # (out_b, h) awaiting readback
                pid = nc.gpsimd.partition_id()
                roff = (pid % 4) * TPT

                def readback(h):
                    out_b = ag_out[h]
                    for r in range(4):
                        nc.gpsimd.dma_start(
                            yf_h[h][:, r * TPT:(r + 1) * TPT],
                            out_b[r * 128:(r + 1) * 128, bass.ds(roff, TPT)])

                attn_psum = tc.tile_pool(name="pa", bufs=2, space="PSUM")
                pa = attn_psum.__enter__()
                py_cm = tc.tile_pool(name="py", bufs=2, space="PSUM")
                py = py_cm.__enter__()
                pd_cm = tc.tile_pool(name="pd", bufs=2, space="PSUM")
                pd = pd_cm.__enter__()

                for h in range(HL):
                    # prefetch this head's output-projection weight panels
                    for j in range(4):
                        et = 4 * j + h
                        nc.sync.dma_start(
                            wo_h[h][:, j * S:(j + 1) * S],
                            wo_d[et * 128:(et + 1) * 128, :])
                    yT = ytpool.tile([128, S], BF16, tag="yT", name="yT")
                    for qc in range(NQC):
                        nblk = 4 * (qc + 1)
                        nfull = 4 * qc
                        npair = nblk // 2
                        ps_y = py.tile([128, QC], F32, tag="y", name="ps_y")
                        acc = accpool.tile([128, QC], BF16, tag="acc", name="acc")
                        qsl = qT[:, h * S + qc * QC: h * S + (qc + 1) * QC]

                        pend_av = []  # (ex2, ga) pairs awaiting AV matmuls

                        def emit_av(ex2, ga):
                            for g, off in ((ga, 0), (ga + 1, QC)):
                                w0 = max(0, g - nfull) * KB
                                nc.tensor.matmul(
                                    ps_y[:, w0:QC],
                                    V[:, g * 128:(g + 1) * 128],
                                    ex2[:, off + w0: off + QC],
                                    start=(g == 0), stop=(g == nblk - 1))

                        for p in range(npair):
                            ga = 2 * p
                            pa2 = pa.tile([128, 2 * QC], F32, tag="s", name="pa2")
                            nc.tensor.matmul(
                                pa2[:, 0:QC],
                                kT[:, ga * KB:(ga + 1) * KB],
                                qsl, start=True, stop=True)
                            nc.tensor.matmul(
                                pa2[:, QC:2 * QC],
                                kT[:, (ga + 1) * KB:(ga + 2) * KB],
                                qsl, start=True, stop=True)
                            ex2 = epool.tile([128, 2 * QC], BF16, tag="e",
                                             name="ex2")
                            nc.scalar.activation(
                                ex2[:], pa2[:],
                                mybir.ActivationFunctionType.Exp, bias=b0[:])
                            if ga >= nfull:  # diagonal pair: causal mask
                                ta = ga - nfull
                                nc.vector.tensor_mul(
                                    ex2[:], ex2[:],
                                    msk[:, ta * QC:(ta + 2) * QC])
                            # denominator accumulation on DVE (bf16)
                            if p == 0:
                                nc.vector.tensor_add(
                                    acc[:], ex2[:, 0:QC], ex2[:, QC:2 * QC])
                            else:
                                ap = epool.tile([128, QC], BF16, tag="ap",
                                                name="accp")
                                nc.vector.tensor_add(
                                    ap[:], ex2[:, 0:QC], ex2[:, QC:2 * QC])
                                nc.vector.tensor_add(acc[:], acc[:], ap[:])
                            pend_av.append((ex2, ga))
                            if len(pend_av) > AVDEPTH:
                                emit_av(*pend_av.pop(0))
                        for args in pend_av:
                            emit_av(*args)

                        # denominator: ones-matmul over the bf16 accumulator
                        den = pd.tile([1, QC], F32, tag="den", name="den")
                        nc.tensor.matmul(den[:], ones[:], acc[:],
                                         start=True, stop=True)
                        rec1 = rspool.tile([1, QC], F32, tag="rc1", name="rec1")
                        nc.vector.reciprocal_approx_fast(rec1[:], den[:])
                        rec = rspool.tile([128, QC], F32, tag="rec", name="rec")
                        nc.gpsimd.partition_broadcast(rec[:], rec1[:])
                        nc.vector.tensor_mul(
                            yT[:, qc * QC:(qc + 1) * QC], ps_y[:], rec[:])

                    # per-head AllGather of y, overlapped with later heads
                    in_b = dpool.tile([128, S], BF16, tag=f"agin{h}",
                                      name=f"agin{h}")
                    out_b = dpool.tile([4 * 128, S], BF16, tag=f"agout{h}",
                                       name=f"agout{h}")
                    nc.sync.dma_start(in_b[:], yT[:])
                    nc.gpsimd.collective_compute(
                        "AllGather", mybir.AluOpType.bypass,
                        replica_groups=RG,
                        ins=[in_b.opt()], outs=[out_b.opt()])
                    ag_out.append(out_b)

                for h in range(HL):
                    readback(h)

                # release attention PSUM banks, open outproj pool
                pd_cm.__exit__(None, None, None)
                py_cm.__exit__(None, None, None)
                attn_psum.__exit__(None, None, None)
                po_cm = tc.tile_pool(name="po", bufs=4, space="PSUM")
                po = po_cm.__enter__()

                # ---- output projection: one pass per head, accumulated ----
                for h in range(HL):
                    last = (h == HL - 1)
                    for ot in range(NDT):
                        ps_o = po.tile([128, TPT], F32, tag="o", name="ps_o")
                        for j in range(4):
                            nc.tensor.matmul(
                                ps_o[:],
                                wo_h[h][:, j * S + ot * 128: j * S + ot * 128 + 128],
                                yf_h[h][:, j * TPT:(j + 1) * TPT],
                                start=(j == 0), stop=(j == 3))
                        psl = part[:, ot * TPT:(ot + 1) * TPT]
                        if h == 0:
                            nc.vector.tensor_copy(psl, ps_o[:])
                        elif not last:
                            nc.vector.tensor_add(psl, psl, ps_o[:])
                        else:
                            ott = otpool.tile([128, TPT], BF16, tag="ot",
                                              name="ott")
                            nc.vector.tensor_add(ott[:], ps_o[:], psl)
                            nc.sync.dma_start(
                                out_d[ot * 128:(ot + 1) * 128, :], ott[:])
                po_cm.__exit__(None, None, None)

    nc.compile()
    return nc


def make_in_maps(x, freqs_cis, wqkv, wo, q_norm_w, k_norm_w, S):
    """Host-side sharding / layout prep. Returns list of 8 input dicts."""
    bf = ml_dtypes.bfloat16
    QC = 512
    KB = 128

    # rope tables: [128, S]; row 2i & 2i+1 carry cos[t, i]; sin signed
    cos = np.asarray(freqs_cis[:S, :, 0], np.float32)   # [S, 64]
    sin = np.asarray(freqs_cis[:S, :, 1], np.float32)
    cosF = np.repeat(cos.T, 2, axis=0).astype(np.float32)      # [128, S]
    sinF = np.repeat(sin.T, 2, axis=0).astype(np.float32)
    sinF[0::2] *= -1.0
    cosF = np.ascontiguousarray(cosF)
    sinF = np.ascontiguousarray(sinF)

    swapP = np.zeros((128, 128), np.float32)
    for i in range(64):
        swapP[2 * i, 2 * i + 1] = 1.0
        swapP[2 * i + 1, 2 * i] = 1.0
    swapP = swapP.astype(bf)
    ident = np.eye(128, dtype=bf)

    # masks [KB, (QC//KB)*QC]: pattern t for the t-th kv block inside the
    # diagonal QC-region: allowed iff (t*KB + r) <= c
    r = np.arange(KB)[:, None]
    c = np.arange(QC)[None, :]
    pats = [((t * KB + r) <= c).astype(np.float32) for t in range(QC // KB)]
    masks = np.concatenate(pats, axis=1).astype(bf)

    qwv = np.asarray(q_norm_w, np.float32).reshape(128, 1)
    kwv = np.asarray(k_norm_w, np.float32).reshape(128, 1)

    woT = np.ascontiguousarray(np.asarray(wo, np.float32).T).astype(bf)

    xTb = []
    for b in range(2):
        xTb.append(np.ascontiguousarray(np.asarray(x[b], np.float32).T)
                   .astype(bf))

    wq = np.asarray(wqkv, np.float32)
    q_sz = NH * HD
    in_maps = []
    for c_id in range(N_CORES):
        b, rk = c_id // 4, c_id % 4
        rows = np.concatenate([
            wq[rk * HL * HD:(rk + 1) * HL * HD],          # 4 q heads
            wq[q_sz + rk * HD: q_sz + (rk + 1) * HD],     # k head
            wq[q_sz + NKV * HD + rk * HD:
               q_sz + NKV * HD + (rk + 1) * HD],          # v head
        ], axis=0)                                        # [768, 2048]
        wslice = np.ascontiguousarray(rows.T).astype(bf)  # [2048, 768]
        in_maps.append({
            "xT": xTb[b], "wslice": wslice, "woT": woT,
            "cosF": cosF, "sinF": sinF, "swapP": swapP,
            "ident": ident, "masks": masks,
            "qw": qwv, "kw": kwv,
        })
    return in_maps


_NC_CACHE = {}


def kernel(x, freqs_cis, mask, wqkv, wo, q_norm_w, k_norm_w):
    x = np.asarray(x)
    S = x.shape[1]
    if S not in _NC_CACHE:
        _NC_CACHE[S] = build_graph(S)
    nc = _NC_CACHE[S]
    in_maps = make_in_maps(x, freqs_cis, wqkv, wo, q_norm_w, k_norm_w, S)
    res = run_bass_kernel_spmd(nc, in_maps, core_ids=list(range(N_CORES)))
    TPT = S // 4
    out = np.empty((2, S, DIM), np.float32)
    for c_id in range(N_CORES):
        b, rk = c_id // 4, c_id % 4
        out[b, rk * TPT:(rk + 1) * TPT, :] = res.results[c_id]["out"].T.astype(np.float32)
    return out


# revision 31
# speedup vs baseline: 1.7593x; 1.0124x over previous
"""Distributed Trainium2 kernel for nn_Attention (dense transformer block:
fused QKV projection + per-head RMSNorm + rotary + causal GQA attention + output
projection), running SPMD on 8 NeuronCores.

Sharding (rank-uniform, no divergent control flow):
  - 8 cores = 2 batch groups x 4 tensor-parallel ranks.
  - Core c: batch b = c // 4, rank r = c % 4.
  - QKV projection + attention are head-sharded: core r computes q heads
    4r..4r+3 and kv head r for ALL tokens of its batch.
  - Per-head AllGather re-shards y from head-split to token-split, overlapped
    with the next head's attention; the output projection then runs locally
    with the full contraction dim in 4 per-head passes (no all-reduce).

Layout tricks:
  - Host pre-transposes x, wqkv, wo so the kernel's matmuls need no on-device
    transposes (except tiny 128x128 PE transposes for V).
  - Rope's even/odd pair swap is a 128x128 permutation matmul in bf16
    (cheap; fp32 matmuls run at quarter rate).
  - Scores are computed transposed [kv, q]; exp is fused into the PSUM->SBUF
    eviction on the ScalarEngine, batched two kv-blocks per activation to
    amortize the ~300-cycle instruction overhead.
  - The softmax denominator is accumulated on the GpSimd engine (tensor_add
    over the exp tiles) and reduced across partitions with
    partition_all_reduce -- zero TensorEngine cost.
  - RMSNorm's sum-of-squares also uses partition_all_reduce instead of a
    ones-vector matmul; the 1/sqrt(head_dim) score scale folds into the
    q-side scalar.
  - All big matmuls run in bf16 with f32 PSUM accumulation.
"""

import numpy as np
import ml_dtypes

import concourse.bass as bass
import concourse.bass_isa as bass_isa
import concourse.mybir as mybir
import concourse.tile as tile
from concourse import bacc
from concourse.bass_utils import run_bass_kernel_spmd

BF16 = mybir.dt.bfloat16
F32 = mybir.dt.float32

DIM = 2048
NH = 16
NKV = 4
HD = 128
EPS = 1e-5
N_CORES = 8
RG = [[0, 1, 2, 3], [4, 5, 6, 7]]  # per-batch tensor-parallel groups

HL = NH // NKV  # q heads per core (= GQA group size) = 4
EW = HL * HD + 2 * HD  # wqkv column-slice width per core = 768
NDT = DIM // 128  # contraction tiles = 16


def build_graph(S):
    """Build + compile the SPMD graph for sequence length S. Returns nc."""
    TPT = S // 4       # tokens per core after the gather (output rows per core)
    TCW = S // 4       # token chunk width for phase 1 (moving dim <= 512)
    NTT = S // TCW     # number of token chunks = 4
    QC = 512           # attention q-chunk width
    KB = 128           # kv block size
    NQC = S // QC      # q chunks per head
    NB = S // 128      # 128-token blocks (for V layout)
    AVDEPTH = 3        # kv-block pairs the AV matmuls trail the score matmuls

    nc = bacc.Bacc("TRN2", target_bir_lowering=False, debug=False,
                   num_devices=N_CORES)

    # ---- DRAM I/O ----
    xT_d = nc.dram_tensor("xT", [DIM, S], BF16, kind="ExternalInput")
    w_d = nc.dram_tensor("wslice", [DIM, EW], BF16, kind="ExternalInput")
    wo_d = nc.dram_tensor("woT", [DIM, DIM], BF16, kind="ExternalInput")
    cos_d = nc.dram_tensor("cosF", [128, S], F32, kind="ExternalInput")
    sin_d = nc.dram_tensor("sinF", [128, S], F32, kind="ExternalInput")
    swp_d = nc.dram_tensor("swapP", [128, 128], BF16, kind="ExternalInput")
    idn_d = nc.dram_tensor("ident", [128, 128], BF16, kind="ExternalInput")
    msk_d = nc.dram_tensor("masks", [KB, (QC // KB) * QC], BF16, kind="ExternalInput")
    qw_d = nc.dram_tensor("qw", [128, 1], F32, kind="ExternalInput")
    kw_d = nc.dram_tensor("kw", [128, 1], F32, kind="ExternalInput")
    out_d = nc.dram_tensor("out", [DIM, TPT], BF16, kind="ExternalOutput")

    with tile.TileContext(nc) as tc:
        with tc.tile_pool(name="const", bufs=1) as cpool, \
             tc.tile_pool(name="big", bufs=1) as bigpool, \
             tc.tile_pool(name="dram", bufs=1, space="DRAM") as dpool:

            # constants (gpsimd queue: keep the sync queue free for weights)
            swp = cpool.tile([128, 128], BF16, tag="swp")
            nc.gpsimd.dma_start(swp[:], swp_d[:])
            idn = cpool.tile([128, 128], BF16, tag="idn")
            nc.gpsimd.dma_start(idn[:], idn_d[:])
            msk = cpool.tile([KB, (QC // KB) * QC], BF16, tag="msk")
            nc.gpsimd.dma_start(msk[:], msk_d[:])
            qw = cpool.tile([128, 1], F32, tag="qw")
            nc.gpsimd.dma_start(qw[:], qw_d[:])
            kw = cpool.tile([128, 1], F32, tag="kw")
            nc.gpsimd.dma_start(kw[:], kw_d[:])
            ones = cpool.tile([128, 1], BF16, tag="ones")
            nc.vector.memset(ones[:], 1.0)
            onec = cpool.tile([1, 128], BF16, tag="onec")
            nc.vector.memset(onec[:], 1.0)
            b0 = cpool.tile([128, 1], F32, tag="b0")
            nc.vector.memset(b0[:], 0.0)
            bq = cpool.tile([1, 1], F32, tag="bq")
            nc.vector.memset(bq[:], float(HD * EPS))
            bk = cpool.tile([1, 1], F32, tag="bk")
            nc.vector.memset(bk[:], float(EPS))

            # long-lived activations
            qT = bigpool.tile([128, HL * S], BF16, tag="qT")
            kT = bigpool.tile([128, S], BF16, tag="kT")
            V = bigpool.tile([128, S], BF16, tag="V")   # [tok%128, blk*128+d]

            # ---------------- Phase 1: QKV + norm + rope ----------------
            with tc.tile_pool(name="wq", bufs=1) as wpool, \
                 tc.tile_pool(name="x", bufs=1) as xpool, \
                 tc.tile_pool(name="cs", bufs=2) as cspool, \
                 tc.tile_pool(name="scr", bufs=2) as scr, \
                 tc.tile_pool(name="smol", bufs=2) as smol, \
                 tc.tile_pool(name="p1", bufs=3, space="PSUM") as p1, \
                 tc.tile_pool(name="psw", bufs=2, space="PSUM") as psw, \
                 tc.tile_pool(name="pss", bufs=1, space="PSUM") as pss, \
                 tc.tile_pool(name="pvt", bufs=2, space="PSUM") as pvt:

                # full wqkv slice, staged once: [128, dt*EW + e]
                w_sb = wpool.tile([128, NDT * EW], BF16, tag="w")
                # full x, staged once with large contiguous DMAs
                xfull = xpool.tile([128, NDT * S], BF16, tag="x")

                def process_qk(ps, et, tt, cos_t, sin_t):
                    is_q = et < HL
                    # sum of squares over head_dim via ones-vector matmul
                    sqv = smol.tile([128, TCW], BF16, tag="sq2", name="sqv")
                    nc.scalar.activation(
                        sqv[:], ps[:],
                        mybir.ActivationFunctionType.Square, bias=b0[:])
                    ss = pss.tile([1, TCW], F32, tag="ss", name="ss")
                    nc.tensor.matmul(ss[:], ones[:], sqv[:],
                                     start=True, stop=True)
                    sq = smol.tile([1, TCW], F32, tag="sqs", name="sq")
                    if is_q:
                        # 1/sqrt(ss + HD*eps) folds the 1/sqrt(HD) score scale
                        nc.scalar.activation(
                            sq[:], ss[:],
                            mybir.ActivationFunctionType.Sqrt,
                            bias=bq[:], scale=1.0)
                    else:
                        nc.scalar.activation(
                            sq[:], ss[:],
                            mybir.ActivationFunctionType.Sqrt,
                            bias=bk[:], scale=1.0 / HD)
                    inv = smol.tile([1, TCW], F32, tag="inv", name="inv")
                    nc.vector.reciprocal_approx_fast(inv[:], sq[:])
                    invb = scr.tile([128, TCW], F32, tag="invb", name="invb")
                    nc.gpsimd.partition_broadcast(invb[:], inv[:])
                    qf = scr.tile([128, TCW], BF16, tag="qf", name="qf")
                    nc.scalar.mul(qf[:], ps[:], (qw if is_q else kw)[:])
                    # rope: pair swap via bf16 permutation matmul, sinF signed
                    sw = psw.tile([128, TCW], F32, tag="sw", name="sw")
                    nc.tensor.matmul(sw[:], swp[:], qf[:],
                                     start=True, stop=True)
                    t1 = scr.tile([128, TCW], F32, tag="t1", name="t1")
                    nc.vector.tensor_mul(t1[:], qf[:], cos_t[:])
                    t2 = scr.tile([128, TCW], F32, tag="t2", name="t2")
                    nc.vector.tensor_mul(t2[:], sw[:], sin_t[:])
                    nc.vector.tensor_add(t1[:], t1[:], t2[:])
                    dst = (qT[:, et * S + tt * TCW: et * S + tt * TCW + TCW]
                           if is_q else
                           kT[:, tt * TCW: tt * TCW + TCW])
                    nc.vector.tensor_mul(dst, t1[:], invb[:])

                def process_v(ps, tt):
                    vb = smol.tile([128, TCW], BF16, tag="vb", name="vb")
                    nc.scalar.copy(vb[:], ps[:])
                    for bb in range(TCW // 128):
                        tp = pvt.tile([128, 128], BF16, tag="tp", name="tp")
                        nc.tensor.transpose(
                            tp[:], vb[:, bb * 128:(bb + 1) * 128], idn[:])
                        blk = tt * (TCW // 128) + bb
                        nc.scalar.copy(V[:, blk * 128:(blk + 1) * 128], tp[:])

                pending = None  # (psum, et, tt, cos_t, sin_t)
                for dt in range(NDT):  # first token chunk + weights, need-order
                    nc.sync.dma_start(
                        w_sb[:, dt * EW:(dt + 1) * EW],
                        w_d[dt * 128:(dt + 1) * 128, :])
                    nc.scalar.dma_start(
                        xfull[:, dt * S:dt * S + TCW],
                        xT_d[dt * 128:(dt + 1) * 128, 0:TCW])
                for dt in range(NDT):  # bulk of x with large contiguous DMAs
                    eng = nc.scalar if dt % 2 == 0 else nc.gpsimd
                    eng.dma_start(
                        xfull[:, dt * S + TCW:(dt + 1) * S],
                        xT_d[dt * 128:(dt + 1) * 128, TCW:])
                for tt in range(NTT):
                    cos_t = cspool.tile([128, TCW], F32, tag="cos")
                    nc.sync.dma_start(cos_t[:], cos_d[:, tt * TCW:(tt + 1) * TCW])
                    sin_t = cspool.tile([128, TCW], F32, tag="sin")
                    nc.sync.dma_start(sin_t[:], sin_d[:, tt * TCW:(tt + 1) * TCW])

                    for et in range(HL + 2):
                        ps = p1.tile([128, TCW], F32, tag="ps")
                        for dt in range(NDT):
                            nc.tensor.matmul(
                                ps[:],
                                w_sb[:, dt * EW + et * 128:dt * EW + (et + 1) * 128],
                                xfull[:, dt * S + tt * TCW:
                                      dt * S + (tt + 1) * TCW],
                                start=(dt == 0), stop=(dt == NDT - 1),
                            )
                        # process the PREVIOUS tile now: its cross-engine
                        # waits overlap this tile's matmul group
                        if pending is not None:
                            pps, pet, ptt, pc, psn_ = pending
                            if pet < HL + 1:
                                process_qk(pps, pet, ptt, pc, psn_)
                            else:
                                process_v(pps, ptt)
                        pending = (ps, et, tt, cos_t, sin_t)
                pps, pet, ptt, pc, psn_ = pending
                if pet < HL + 1:
                    process_qk(pps, pet, ptt, pc, psn_)
                else:
                    process_v(pps, ptt)

            # ---------------- Phase 2: causal attention + outproj ----------------
            with tc.tile_pool(name="wo", bufs=1) as wopool, \
                 tc.tile_pool(name="part", bufs=1) as partpool, \
                 tc.tile_pool(name="yf", bufs=1) as yfpool, \
                 tc.tile_pool(name="yt", bufs=2) as ytpool, \
                 tc.tile_pool(name="acc", bufs=2) as accpool, \
                 tc.tile_pool(name="exp", bufs=8) as epool, \
                 tc.tile_pool(name="rs", bufs=2) as rspool, \
                 tc.tile_pool(name="ot", bufs=2) as otpool:

                part = partpool.tile([128, NDT * TPT], F32, tag="part")
                wo_h = [wopool.tile([128, 4 * S], BF16, tag=f"wo{h}",
                                    name=f"wo{h}")
                        for h in range(HL)]
                yf_h = [yfpool.tile([128, 4 * TPT], BF16, tag=f"yf{h}",
                                    name=f"yf{h}")
                        for h in range(HL)]
                ag_out = []  # (out_b, h) awaiting readback
                pid = nc.gpsimd.partition_id()
                roff = (pid % 4) * TPT

                def readback(h):
                    out_b = ag_out[h]
                    for r in range(4):
                        nc.gpsimd.dma_start(
                            yf_h[h][:, r * TPT:(r + 1) * TPT],
                            out_b[r * 128:(r + 1) * 128, bass.ds(roff, TPT)])

                attn_psum = tc.tile_pool(name="pa", bufs=2, space="PSUM")
                pa = attn_psum.__enter__()
                py_cm = tc.tile_pool(name="py", bufs=2, space="PSUM")
                py = py_cm.__enter__()
                pd_cm = tc.tile_pool(name="pd", bufs=1, space="PSUM")
                pd = pd_cm.__enter__()

                for h in range(HL):
                    # prefetch this head's output-projection weight panels
                    for j in range(4):
                        et = 4 * j + h
                        nc.sync.dma_start(
                            wo_h[h][:, j * S:(j + 1) * S],
                            wo_d[et * 128:(et + 1) * 128, :])
                    yT = ytpool.tile([128, S], BF16, tag="yT", name="yT")
                    for qc in range(NQC):
                        nblk = 4 * (qc + 1)
                        nfull = 4 * qc
                        npair = nblk // 2
                        ps_y = py.tile([128, QC], F32, tag="y", name="ps_y")
                        acc = accpool.tile([128, QC], BF16, tag="acc", name="acc")
                        qsl = qT[:, h * S + qc * QC: h * S + (qc + 1) * QC]

                        pend_av = []  # (ex2, ga) pairs awaiting AV matmuls

                        def emit_av(ex2, ga):
                            for g, off in ((ga, 0), (ga + 1, QC)):
                                w0 = max(0, g - nfull) * KB
                                nc.tensor.matmul(
                                    ps_y[:, w0:QC],
                                    V[:, g * 128:(g + 1) * 128],
                                    ex2[:, off + w0: off + QC],
                                    start=(g == 0), stop=(g == nblk - 1))

                        for p in range(npair):
                            ga = 2 * p
                            pa2 = pa.tile([128, 2 * QC], F32, tag="s", name="pa2")
                            nc.tensor.matmul(
                                pa2[:, 0:QC],
                                kT[:, ga * KB:(ga + 1) * KB],
                                qsl, start=True, stop=True)
                            nc.tensor.matmul(
                                pa2[:, QC:2 * QC],
                                kT[:, (ga + 1) * KB:(ga + 2) * KB],
                                qsl, start=True, stop=True)
                            ex2 = epool.tile([128, 2 * QC], BF16, tag="e",
                                             name="ex2")
                            nc.scalar.activation(
                                ex2[:], pa2[:],
                                mybir.ActivationFunctionType.Exp, bias=b0[:])
                            if ga >= nfull:  # diagonal pair: causal mask
                                ta = ga - nfull
                                nc.vector.tensor_mul(
                                    ex2[:], ex2[:],
                                    msk[:, ta * QC:(ta + 2) * QC])
                            # denominator accumulation on DVE (bf16)
                            if p == 0:
                                nc.vector.tensor_add(
                                    acc[:], ex2[:, 0:QC], ex2[:, QC:2 * QC])
                            else:
                                ap = epool.tile([128, QC], BF16, tag="ap",
                                                name="accp")
                                nc.vector.tensor_add(
                                    ap[:], ex2[:, 0:QC], ex2[:, QC:2 * QC])
                                nc.vector.tensor_add(acc[:], acc[:], ap[:])
                            pend_av.append((ex2, ga))
                            if len(pend_av) > AVDEPTH:
                                emit_av(*pend_av.pop(0))
                        for args in pend_av:
                            emit_av(*args)

                        # denominator: ones-matmul over the bf16 accumulator;
                        # reciprocal broadcast back to 128 partitions via a
                        # K=1 matmul (keeps attention off the gpsimd queue)
                        den = pd.tile([1, QC], F32, tag="den", name="den")
                        nc.tensor.matmul(den[:], ones[:], acc[:],
                                         start=True, stop=True)
                        rec1 = rspool.tile([1, QC], F32, tag="rc1", name="rec1")
                        nc.vector.reciprocal_approx_fast(rec1[:], den[:])
                        rc16 = rspool.tile([1, QC], BF16, tag="rc6",
                                           name="rc16")
                        nc.vector.tensor_copy(rc16[:], rec1[:])
                        rec = pd.tile([128, QC], F32, tag="bc", name="rec")
                        nc.tensor.matmul(rec[:], onec[:], rc16[:],
                                         start=True, stop=True)
                        rsb = rspool.tile([128, QC], F32, tag="rsb",
                                          name="rsb")
                        nc.vector.tensor_copy(rsb[:], rec[:])
                        nc.vector.tensor_mul(
                            yT[:, qc * QC:(qc + 1) * QC], ps_y[:], rsb[:])

                    # per-head AllGather of y, overlapped with later heads
                    in_b = dpool.tile([128, S], BF16, tag=f"agin{h}",
                                      name=f"agin{h}")
                    out_b = dpool.tile([4 * 128, S], BF16, tag=f"agout{h}",
                                       name=f"agout{h}")
                    nc.sync.dma_start(in_b[:], yT[:])
                    nc.gpsimd.collective_compute(
                        "AllGather", mybir.AluOpType.bypass,
                        replica_groups=RG,
                        ins=[in_b.opt()], outs=[out_b.opt()])
                    ag_out.append(out_b)

                for h in range(HL):
                    readback(h)

                # release attention PSUM banks, open outproj pool
                pd_cm.__exit__(None, None, None)
                py_cm.__exit__(None, None, None)
                attn_psum.__exit__(None, None, None)
                po_cm = tc.tile_pool(name="po", bufs=4, space="PSUM")
                po = po_cm.__enter__()

                # ---- output projection: one pass per head, accumulated ----
                for h in range(HL):
                    last = (h == HL - 1)
                    for ot in range(NDT):
                        ps_o = po.tile([128, TPT], F32, tag="o", name="ps_o")
                        for j in range(4):
                            nc.tensor.matmul(
                                ps_o[:],
                                wo_h[h][:, j * S + ot * 128: j * S + ot * 128 + 128],
                                yf_h[h][:, j * TPT:(j + 1) * TPT],
                                start=(j == 0), stop=(j == 3))
                        psl = part[:, ot * TPT:(ot + 1) * TPT]
                        if h == 0:
                            nc.vector.tensor_copy(psl, ps_o[:])
                        elif not last:
                            nc.vector.tensor_add(psl, psl, ps_o[:])
                        else:
                            ott = otpool.tile([128, TPT], BF16, tag="ot",
                                              name="ott")
                            nc.vector.tensor_add(ott[:], ps_o[:], psl)
                            nc.sync.dma_start(
                                out_d[ot * 128:(ot + 1) * 128, :], ott[:])
                po_cm.__exit__(None, None, None)

    nc.compile()
    return nc


def make_in_maps(x, freqs_cis, wqkv, wo, q_norm_w, k_norm_w, S):
    """Host-side sharding / layout prep. Returns list of 8 input dicts."""
    bf = ml_dtypes.bfloat16
    QC = 512
    KB = 128

    # rope tables: [128, S]; row 2i & 2i+1 carry cos[t, i]; sin signed
    cos = np.asarray(freqs_cis[:S, :, 0], np.float32)   # [S, 64]
    sin = np.asarray(freqs_cis[:S, :, 1], np.float32)
    cosF = np.repeat(cos.T, 2, axis=0).astype(np.float32)      # [128, S]
    sinF = np.repeat(sin.T, 2, axis=0).astype(np.float32)
    sinF[0::2] *= -1.0
    cosF = np.ascontiguousarray(cosF)
    sinF = np.ascontiguousarray(sinF)

    swapP = np.zeros((128, 128), np.float32)
    for i in range(64):
        swapP[2 * i, 2 * i + 1] = 1.0
        swapP[2 * i + 1, 2 * i] = 1.0
    swapP = swapP.astype(bf)
    ident = np.eye(128, dtype=bf)

    # masks [KB, (QC//KB)*QC]: pattern t for the t-th kv block inside the
    # diagonal QC-region: allowed iff (t*KB + r) <= c
    r = np.arange(KB)[:, None]
    c = np.arange(QC)[None, :]
    pats = [((t * KB + r) <= c).astype(np.float32) for t in range(QC // KB)]
    masks = np.concatenate(pats, axis=1).astype(bf)

    qwv = np.asarray(q_norm_w, np.float32).reshape(128, 1)
    kwv = np.asarray(k_norm_w, np.float32).reshape(128, 1)

    woT = np.ascontiguousarray(np.asarray(wo, np.float32).T).astype(bf)

    xTb = []
    for b in range(2):
        xTb.append(np.ascontiguousarray(np.asarray(x[b], np.float32).T)
                   .astype(bf))

    wq = np.asarray(wqkv, np.float32)
    q_sz = NH * HD
    in_maps = []
    for c_id in range(N_CORES):
        b, rk = c_id // 4, c_id % 4
        rows = np.concatenate([
            wq[rk * HL * HD:(rk + 1) * HL * HD],          # 4 q heads
            wq[q_sz + rk * HD: q_sz + (rk + 1) * HD],     # k head
            wq[q_sz + NKV * HD + rk * HD:
               q_sz + NKV * HD + (rk + 1) * HD],          # v head
        ], axis=0)                                        # [768, 2048]
        wslice = np.ascontiguousarray(rows.T).astype(bf)  # [2048, 768]
        in_maps.append({
            "xT": xTb[b], "wslice": wslice, "woT": woT,
            "cosF": cosF, "sinF": sinF, "swapP": swapP,
            "ident": ident, "masks": masks,
            "qw": qwv, "kw": kwv,
        })
    return in_maps


_NC_CACHE = {}


def kernel(x, freqs_cis, mask, wqkv, wo, q_norm_w, k_norm_w):
    x = np.asarray(x)
    S = x.shape[1]
    if S not in _NC_CACHE:
        _NC_CACHE[S] = build_graph(S)
    nc = _NC_CACHE[S]
    in_maps = make_in_maps(x, freqs_cis, wqkv, wo, q_norm_w, k_norm_w, S)
    res = run_bass_kernel_spmd(nc, in_maps, core_ids=list(range(N_CORES)))
    TPT = S // 4
    out = np.empty((2, S, DIM), np.float32)
    for c_id in range(N_CORES):
        b, rk = c_id // 4, c_id % 4
        out[b, rk * TPT:(rk + 1) * TPT, :] = res.results[c_id]["out"].T.astype(np.float32)
    return out


# revision 34
# speedup vs baseline: 1.7911x; 1.0181x over previous
"""Distributed Trainium2 kernel for nn_Attention (dense transformer block:
fused QKV projection + per-head RMSNorm + rotary + causal GQA attention + output
projection), running SPMD on 8 NeuronCores.

Sharding (rank-uniform, no divergent control flow):
  - 8 cores = 2 batch groups x 4 tensor-parallel ranks.
  - Core c: batch b = c // 4, rank r = c % 4.
  - QKV projection + attention are head-sharded: core r computes q heads
    4r..4r+3 and kv head r for ALL tokens of its batch.
  - Per-head AllGather re-shards y from head-split to token-split, overlapped
    with the next head's attention; the output projection then runs locally
    with the full contraction dim in 4 per-head passes (no all-reduce).

Layout tricks:
  - Host pre-transposes x, wqkv, wo so the kernel's matmuls need no on-device
    transposes (except tiny 128x128 PE transposes for V).
  - Rope's even/odd pair swap is a 128x128 permutation matmul in bf16
    (cheap; fp32 matmuls run at quarter rate).
  - Scores are computed transposed [kv, q]; exp is fused into the PSUM->SBUF
    eviction on the ScalarEngine, batched two kv-blocks per activation to
    amortize the ~300-cycle instruction overhead.
  - The softmax denominator is accumulated on the GpSimd engine (tensor_add
    over the exp tiles) and reduced across partitions with
    partition_all_reduce -- zero TensorEngine cost.
  - RMSNorm's sum-of-squares also uses partition_all_reduce instead of a
    ones-vector matmul; the 1/sqrt(head_dim) score scale folds into the
    q-side scalar.
  - All big matmuls run in bf16 with f32 PSUM accumulation.
"""

import numpy as np
import ml_dtypes

import concourse.bass as bass
import concourse.bass_isa as bass_isa
import concourse.mybir as mybir
import concourse.tile as tile
from concourse import bacc
from concourse.bass_utils import run_bass_kernel_spmd

BF16 = mybir.dt.bfloat16
F32 = mybir.dt.float32

DIM = 2048
NH = 16
NKV = 4
HD = 128
EPS = 1e-5
N_CORES = 8
RG = [[0, 1, 2, 3], [4, 5, 6, 7]]  # per-batch tensor-parallel groups

HL = NH // NKV  # q heads per core (= GQA group size) = 4
EW = HL * HD + 2 * HD  # wqkv column-slice width per core = 768
NDT = DIM // 128  # contraction tiles = 16


def build_graph(S):
    """Build + compile the SPMD graph for sequence length S. Returns nc."""
    TPT = S // 4       # tokens per core after the gather (output rows per core)
    TCW = S // 4       # token chunk width for phase 1 (moving dim <= 512)
    NTT = S // TCW     # number of token chunks = 4
    QC = 512           # attention q-chunk width
    KB = 128           # kv block size
    NQC = S // QC      # q chunks per head
    NB = S // 128      # 128-token blocks (for V layout)
    AVDEPTH = 3        # kv-block pairs the AV matmuls trail the score matmuls

    nc = bacc.Bacc("TRN2", target_bir_lowering=False, debug=False,
                   num_devices=N_CORES)

    # ---- DRAM I/O ----
    xT_d = nc.dram_tensor("xT", [DIM, S], BF16, kind="ExternalInput")
    w_d = nc.dram_tensor("wslice", [DIM, EW], BF16, kind="ExternalInput")
    wo_d = nc.dram_tensor("woT", [DIM, DIM], BF16, kind="ExternalInput")
    cos_d = nc.dram_tensor("cosF", [128, S], F32, kind="ExternalInput")
    sin_d = nc.dram_tensor("sinF", [128, S], F32, kind="ExternalInput")
    swp_d = nc.dram_tensor("swapP", [128, 128], BF16, kind="ExternalInput")
    idn_d = nc.dram_tensor("ident", [128, 128], BF16, kind="ExternalInput")
    msk_d = nc.dram_tensor("masks", [KB, (QC // KB) * QC], BF16, kind="ExternalInput")
    qw_d = nc.dram_tensor("qw", [128, 1], F32, kind="ExternalInput")
    kw_d = nc.dram_tensor("kw", [128, 1], F32, kind="ExternalInput")
    out_d = nc.dram_tensor("out", [DIM, TPT], BF16, kind="ExternalOutput")

    with tile.TileContext(nc) as tc:
        with tc.tile_pool(name="const", bufs=1) as cpool, \
             tc.tile_pool(name="big", bufs=1) as bigpool, \
             tc.tile_pool(name="dram", bufs=1, space="DRAM") as dpool:

            # constants (gpsimd queue: keep the sync queue free for weights)
            swp = cpool.tile([128, 128], BF16, tag="swp")
            nc.gpsimd.dma_start(swp[:], swp_d[:])
            idn = cpool.tile([128, 128], BF16, tag="idn")
            nc.gpsimd.dma_start(idn[:], idn_d[:])
            msk = cpool.tile([KB, (QC // KB) * QC], BF16, tag="msk")
            nc.gpsimd.dma_start(msk[:], msk_d[:])
            qw = cpool.tile([128, 1], F32, tag="qw")
            nc.gpsimd.dma_start(qw[:], qw_d[:])
            kw = cpool.tile([128, 1], F32, tag="kw")
            nc.gpsimd.dma_start(kw[:], kw_d[:])
            ones = cpool.tile([128, 1], BF16, tag="ones")
            nc.vector.memset(ones[:], 1.0)
            onec = cpool.tile([1, 128], BF16, tag="onec")
            nc.vector.memset(onec[:], 1.0)
            b0 = cpool.tile([128, 1], F32, tag="b0")
            nc.vector.memset(b0[:], 0.0)
            bq = cpool.tile([1, 1], F32, tag="bq")
            nc.vector.memset(bq[:], float(HD * EPS))
            bk = cpool.tile([1, 1], F32, tag="bk")
            nc.vector.memset(bk[:], float(EPS))

            # long-lived activations
            qT = bigpool.tile([128, HL * S], BF16, tag="qT")
            kT = bigpool.tile([128, S], BF16, tag="kT")
            V = bigpool.tile([128, S], BF16, tag="V")   # [tok%128, blk*128+d]

            # ---------------- Phase 1: QKV + norm + rope ----------------
            with tc.tile_pool(name="wq", bufs=1) as wpool, \
                 tc.tile_pool(name="x", bufs=1) as xpool, \
                 tc.tile_pool(name="cs", bufs=2) as cspool, \
                 tc.tile_pool(name="scr", bufs=2) as scr, \
                 tc.tile_pool(name="smol", bufs=2) as smol, \
                 tc.tile_pool(name="p1", bufs=3, space="PSUM") as p1, \
                 tc.tile_pool(name="psw", bufs=2, space="PSUM") as psw, \
                 tc.tile_pool(name="pss", bufs=1, space="PSUM") as pss, \
                 tc.tile_pool(name="pvt", bufs=2, space="PSUM") as pvt:

                # full wqkv slice, staged once: [128, dt*EW + e]
                w_sb = wpool.tile([128, NDT * EW], BF16, tag="w")
                # full x, staged once with large contiguous DMAs
                xfull = xpool.tile([128, NDT * S], BF16, tag="x")

                def process_qk(ps, et, tt, cos_t, sin_t):
                    is_q = et < HL
                    # sum of squares over head_dim via ones-vector matmul
                    sqv = smol.tile([128, TCW], BF16, tag="sq2", name="sqv")
                    nc.scalar.activation(
                        sqv[:], ps[:],
                        mybir.ActivationFunctionType.Square, bias=b0[:])
                    ss = pss.tile([1, TCW], F32, tag="ss", name="ss")
                    nc.tensor.matmul(ss[:], ones[:], sqv[:],
                                     start=True, stop=True)
                    sq = smol.tile([1, TCW], F32, tag="sqs", name="sq")
                    if is_q:
                        # 1/sqrt(ss + HD*eps) folds the 1/sqrt(HD) score scale
                        nc.scalar.activation(
                            sq[:], ss[:],
                            mybir.ActivationFunctionType.Sqrt,
                            bias=bq[:], scale=1.0)
                    else:
                        nc.scalar.activation(
                            sq[:], ss[:],
                            mybir.ActivationFunctionType.Sqrt,
                            bias=bk[:], scale=1.0 / HD)
                    inv = smol.tile([1, TCW], F32, tag="inv", name="inv")
                    nc.vector.reciprocal_approx_fast(inv[:], sq[:])
                    invb = scr.tile([128, TCW], F32, tag="invb", name="invb")
                    nc.gpsimd.partition_broadcast(invb[:], inv[:])
                    qf = scr.tile([128, TCW], BF16, tag="qf", name="qf")
                    nc.scalar.mul(qf[:], ps[:], (qw if is_q else kw)[:])
                    # rope: pair swap via bf16 permutation matmul, sinF signed
                    sw = psw.tile([128, TCW], F32, tag="sw", name="sw")
                    nc.tensor.matmul(sw[:], swp[:], qf[:],
                                     start=True, stop=True)
                    t1 = scr.tile([128, TCW], F32, tag="t1", name="t1")
                    nc.vector.tensor_mul(t1[:], qf[:], cos_t[:])
                    t2 = scr.tile([128, TCW], F32, tag="t2", name="t2")
                    nc.vector.tensor_mul(t2[:], sw[:], sin_t[:])
                    nc.vector.tensor_add(t1[:], t1[:], t2[:])
                    dst = (qT[:, et * S + tt * TCW: et * S + tt * TCW + TCW]
                           if is_q else
                           kT[:, tt * TCW: tt * TCW + TCW])
                    nc.vector.tensor_mul(dst, t1[:], invb[:])

                def process_v(ps, tt):
                    vb = smol.tile([128, TCW], BF16, tag="vb", name="vb")
                    nc.scalar.copy(vb[:], ps[:])
                    for bb in range(TCW // 128):
                        tp = pvt.tile([128, 128], BF16, tag="tp", name="tp")
                        nc.tensor.transpose(
                            tp[:], vb[:, bb * 128:(bb + 1) * 128], idn[:])
                        blk = tt * (TCW // 128) + bb
                        nc.scalar.copy(V[:, blk * 128:(blk + 1) * 128], tp[:])

                pending = None  # (psum, et, tt, cos_t, sin_t)
                for dt in range(NDT):  # first token chunk + weights, need-order
                    nc.sync.dma_start(
                        w_sb[:, dt * EW:(dt + 1) * EW],
                        w_d[dt * 128:(dt + 1) * 128, :])
                    nc.scalar.dma_start(
                        xfull[:, dt * S:dt * S + TCW],
                        xT_d[dt * 128:(dt + 1) * 128, 0:TCW])
                for dt in range(NDT):  # bulk of x with large contiguous DMAs
                    eng = (nc.scalar, nc.gpsimd, nc.sync)[dt % 3]
                    eng.dma_start(
                        xfull[:, dt * S + TCW:(dt + 1) * S],
                        xT_d[dt * 128:(dt + 1) * 128, TCW:])
                for tt in range(NTT):
                    cos_t = cspool.tile([128, TCW], F32, tag="cos")
                    nc.sync.dma_start(cos_t[:], cos_d[:, tt * TCW:(tt + 1) * TCW])
                    sin_t = cspool.tile([128, TCW], F32, tag="sin")
                    nc.sync.dma_start(sin_t[:], sin_d[:, tt * TCW:(tt + 1) * TCW])

                    for et in range(HL + 2):
                        ps = p1.tile([128, TCW], F32, tag="ps")
                        for dt in range(NDT):
                            nc.tensor.matmul(
                                ps[:],
                                w_sb[:, dt * EW + et * 128:dt * EW + (et + 1) * 128],
                                xfull[:, dt * S + tt * TCW:
                                      dt * S + (tt + 1) * TCW],
                                start=(dt == 0), stop=(dt == NDT - 1),
                            )
                        # process the PREVIOUS tile now: its cross-engine
                        # waits overlap this tile's matmul group
                        if pending is not None:
                            pps, pet, ptt, pc, psn_ = pending
                            if pet < HL + 1:
                                process_qk(pps, pet, ptt, pc, psn_)
                            else:
                                process_v(pps, ptt)
                        pending = (ps, et, tt, cos_t, sin_t)
                pps, pet, ptt, pc, psn_ = pending
                if pet < HL + 1:
                    process_qk(pps, pet, ptt, pc, psn_)
                else:
                    process_v(pps, ptt)

            # ---------------- Phase 2: causal attention + outproj ----------------
            with tc.tile_pool(name="wo", bufs=1) as wopool, \
                 tc.tile_pool(name="part", bufs=1) as partpool, \
                 tc.tile_pool(name="yf", bufs=1) as yfpool, \
                 tc.tile_pool(name="yt", bufs=2) as ytpool, \
                 tc.tile_pool(name="acc", bufs=2) as accpool, \
                 tc.tile_pool(name="exp", bufs=8) as epool, \
                 tc.tile_pool(name="rs", bufs=2) as rspool, \
                 tc.tile_pool(name="ot", bufs=2) as otpool:

                part = partpool.tile([128, NDT * TPT], F32, tag="part")
                wo_h = [wopool.tile([128, 4 * S], BF16, tag=f"wo{h}",
                                    name=f"wo{h}")
                        for h in range(HL)]
                yf_h = [yfpool.tile([128, 4 * TPT], BF16, tag=f"yf{h}",
                                    name=f"yf{h}")
                        for h in range(HL)]
                ag_out = []  # (out_b, h) awaiting readback
                pid = nc.gpsimd.partition_id()
                roff = (pid % 4) * TPT

                def readback(h):
                    out_b = ag_out[h]
                    for r in range(4):
                        nc.gpsimd.dma_start(
                            yf_h[h][:, r * TPT:(r + 1) * TPT],
                            out_b[r * 128:(r + 1) * 128, bass.ds(roff, TPT)])

                attn_psum = tc.tile_pool(name="pa", bufs=2, space="PSUM")
                pa = attn_psum.__enter__()
                py_cm = tc.tile_pool(name="py", bufs=2, space="PSUM")
                py = py_cm.__enter__()
                pd_cm = tc.tile_pool(name="pd", bufs=1, space="PSUM")
                pd = pd_cm.__enter__()

                # prefetch ALL output-projection weight panels up front so the
                # per-head AllGathers face no competing DMA traffic
                for h in range(HL):
                    for j in range(4):
                        et = 4 * j + h
                        nc.sync.dma_start(
                            wo_h[h][:, j * S:(j + 1) * S],
                            wo_d[et * 128:(et + 1) * 128, :])

                for h in range(HL):
                    yT = ytpool.tile([128, S], BF16, tag="yT", name="yT")
                    in_b = dpool.tile([128, S], BF16, tag=f"agin{h}",
                                      name=f"agin{h}")
                    for qc in range(NQC):
                        nblk = 4 * (qc + 1)
                        nfull = 4 * qc
                        npair = nblk // 2
                        ps_y = py.tile([128, QC], F32, tag="y", name="ps_y")
                        acc = accpool.tile([128, QC], BF16, tag="acc", name="acc")
                        qsl = qT[:, h * S + qc * QC: h * S + (qc + 1) * QC]

                        pend_av = []  # (ex2, ga) pairs awaiting AV matmuls

                        def emit_av(ex2, ga):
                            for g, off in ((ga, 0), (ga + 1, QC)):
                                w0 = max(0, g - nfull) * KB
                                nc.tensor.matmul(
                                    ps_y[:, w0:QC],
                                    V[:, g * 128:(g + 1) * 128],
                                    ex2[:, off + w0: off + QC],
                                    start=(g == 0), stop=(g == nblk - 1))

                        for p in range(npair):
                            ga = 2 * p
                            pa2 = pa.tile([128, 2 * QC], F32, tag="s", name="pa2")
                            nc.tensor.matmul(
                                pa2[:, 0:QC],
                                kT[:, ga * KB:(ga + 1) * KB],
                                qsl, start=True, stop=True)
                            nc.tensor.matmul(
                                pa2[:, QC:2 * QC],
                                kT[:, (ga + 1) * KB:(ga + 2) * KB],
                                qsl, start=True, stop=True)
                            ex2 = epool.tile([128, 2 * QC], BF16, tag="e",
                                             name="ex2")
                            nc.scalar.activation(
                                ex2[:], pa2[:],
                                mybir.ActivationFunctionType.Exp, bias=b0[:])
                            if ga >= nfull:  # diagonal pair: causal mask
                                ta = ga - nfull
                                nc.vector.tensor_mul(
                                    ex2[:], ex2[:],
                                    msk[:, ta * QC:(ta + 2) * QC])
                            # denominator accumulation on DVE (bf16)
                            if p == 0:
                                nc.vector.tensor_add(
                                    acc[:], ex2[:, 0:QC], ex2[:, QC:2 * QC])
                            else:
                                ap = epool.tile([128, QC], BF16, tag="ap",
                                                name="accp")
                                nc.vector.tensor_add(
                                    ap[:], ex2[:, 0:QC], ex2[:, QC:2 * QC])
                                nc.vector.tensor_add(acc[:], acc[:], ap[:])
                            pend_av.append((ex2, ga))
                            if len(pend_av) > AVDEPTH:
                                emit_av(*pend_av.pop(0))
                        for args in pend_av:
                            emit_av(*args)

                        # denominator: ones-matmul over the bf16 accumulator;
                        # reciprocal broadcast back to 128 partitions via a
                        # K=1 matmul (keeps attention off the gpsimd queue)
                        den = pd.tile([1, QC], F32, tag="den", name="den")
                        nc.tensor.matmul(den[:], ones[:], acc[:],
                                         start=True, stop=True)
                        rec1 = rspool.tile([1, QC], F32, tag="rc1", name="rec1")
                        nc.vector.reciprocal_approx_fast(rec1[:], den[:])
                        rc16 = rspool.tile([1, QC], BF16, tag="rc6",
                                           name="rc16")
                        nc.vector.tensor_copy(rc16[:], rec1[:])
                        rec = pd.tile([128, QC], F32, tag="bc", name="rec")
                        nc.tensor.matmul(rec[:], onec[:], rc16[:],
                                         start=True, stop=True)
                        rsb = rspool.tile([128, QC], F32, tag="rsb",
                                          name="rsb")
                        nc.vector.tensor_copy(rsb[:], rec[:])
                        nc.vector.tensor_mul(
                            yT[:, qc * QC:(qc + 1) * QC], ps_y[:], rsb[:])
                        # stage this q-chunk's slice of the gather input now
                        nc.sync.dma_start(
                            in_b[:, qc * QC:(qc + 1) * QC],
                            yT[:, qc * QC:(qc + 1) * QC])

                    # per-head AllGather of y, overlapped with later heads
                    out_b = dpool.tile([4 * 128, S], BF16, tag=f"agout{h}",
                                       name=f"agout{h}")
                    nc.gpsimd.collective_compute(
                        "AllGather", mybir.AluOpType.bypass,
                        replica_groups=RG,
                        ins=[in_b.opt()], outs=[out_b.opt()])
                    ag_out.append(out_b)

                for h in range(HL):
                    readback(h)

                # release attention PSUM banks, open outproj pool
                pd_cm.__exit__(None, None, None)
                py_cm.__exit__(None, None, None)
                attn_psum.__exit__(None, None, None)
                po_cm = tc.tile_pool(name="po", bufs=4, space="PSUM")
                po = po_cm.__enter__()

                # ---- output projection: one pass per head, accumulated ----
                for h in range(HL):
                    last = (h == HL - 1)
                    for ot in range(NDT):
                        ps_o = po.tile([128, TPT], F32, tag="o", name="ps_o")
                        for j in range(4):
                            nc.tensor.matmul(
                                ps_o[:],
                                wo_h[h][:, j * S + ot * 128: j * S + ot * 128 + 128],
                                yf_h[h][:, j * TPT:(j + 1) * TPT],
                                start=(j == 0), stop=(j == 3))
                        psl = part[:, ot * TPT:(ot + 1) * TPT]
                        if h == 0:
                            nc.vector.tensor_copy(psl, ps_o[:])
                        elif not last:
                            nc.vector.tensor_add(psl, psl, ps_o[:])
                        else:
                            ott = otpool.tile([128, TPT], BF16, tag="ot",
                                              name="ott")
                            nc.vector.tensor_add(ott[:], ps_o[:], psl)
                            nc.sync.dma_start(
                                out_d[ot * 128:(ot + 1) * 128, :], ott[:])
                po_cm.__exit__(None, None, None)

    nc.compile()
    return nc


def make_in_maps(x, freqs_cis, wqkv, wo, q_norm_w, k_norm_w, S):
    """Host-side sharding / layout prep. Returns list of 8 input dicts."""
    bf = ml_dtypes.bfloat16
    QC = 512
    KB = 128

    # rope tables: [128, S]; row 2i & 2i+1 carry cos[t, i]; sin signed
    cos = np.asarray(freqs_cis[:S, :, 0], np.float32)   # [S, 64]
    sin = np.asarray(freqs_cis[:S, :, 1], np.float32)
    cosF = np.repeat(cos.T, 2, axis=0).astype(np.float32)      # [128, S]
    sinF = np.repeat(sin.T, 2, axis=0).astype(np.float32)
    sinF[0::2] *= -1.0
    cosF = np.ascontiguousarray(cosF)
    sinF = np.ascontiguousarray(sinF)

    swapP = np.zeros((128, 128), np.float32)
    for i in range(64):
        swapP[2 * i, 2 * i + 1] = 1.0
        swapP[2 * i + 1, 2 * i] = 1.0
    swapP = swapP.astype(bf)
    ident = np.eye(128, dtype=bf)

    # masks [KB, (QC//KB)*QC]: pattern t for the t-th kv block inside the
    # diagonal QC-region: allowed iff (t*KB + r) <= c
    r = np.arange(KB)[:, None]
    c = np.arange(QC)[None, :]
    pats = [((t * KB + r) <= c).astype(np.float32) for t in range(QC // KB)]
    masks = np.concatenate(pats, axis=1).astype(bf)

    qwv = np.asarray(q_norm_w, np.float32).reshape(128, 1)
    kwv = np.asarray(k_norm_w, np.float32).reshape(128, 1)

    woT = np.ascontiguousarray(np.asarray(wo, np.float32).T).astype(bf)

    xTb = []
    for b in range(2):
        xTb.append(np.ascontiguousarray(np.asarray(x[b], np.float32).T)
                   .astype(bf))

    wq = np.asarray(wqkv, np.float32)
    q_sz = NH * HD
    in_maps = []
    for c_id in range(N_CORES):
        b, rk = c_id // 4, c_id % 4
        rows = np.concatenate([
            wq[rk * HL * HD:(rk + 1) * HL * HD],          # 4 q heads
            wq[q_sz + rk * HD: q_sz + (rk + 1) * HD],     # k head
            wq[q_sz + NKV * HD + rk * HD:
               q_sz + NKV * HD + (rk + 1) * HD],          # v head
        ], axis=0)                                        # [768, 2048]
        wslice = np.ascontiguousarray(rows.T).astype(bf)  # [2048, 768]
        in_maps.append({
            "xT": xTb[b], "wslice": wslice, "woT": woT,
            "cosF": cosF, "sinF": sinF, "swapP": swapP,
            "ident": ident, "masks": masks,
            "qw": qwv, "kw": kwv,
        })
    return in_maps


_NC_CACHE = {}


def kernel(x, freqs_cis, mask, wqkv, wo, q_norm_w, k_norm_w):
    x = np.asarray(x)
    S = x.shape[1]
    if S not in _NC_CACHE:
        _NC_CACHE[S] = build_graph(S)
    nc = _NC_CACHE[S]
    in_maps = make_in_maps(x, freqs_cis, wqkv, wo, q_norm_w, k_norm_w, S)
    res = run_bass_kernel_spmd(nc, in_maps, core_ids=list(range(N_CORES)))
    TPT = S // 4
    out = np.empty((2, S, DIM), np.float32)
    for c_id in range(N_CORES):
        b, rk = c_id // 4, c_id % 4
        out[b, rk * TPT:(rk + 1) * TPT, :] = res.results[c_id]["out"].T.astype(np.float32)
    return out


# revision 36
# speedup vs baseline: 1.7932x; 1.0012x over previous
"""Distributed Trainium2 kernel for nn_Attention (dense transformer block:
fused QKV projection + per-head RMSNorm + rotary + causal GQA attention + output
projection), running SPMD on 8 NeuronCores.

Sharding (rank-uniform, no divergent control flow):
  - 8 cores = 2 batch groups x 4 tensor-parallel ranks.
  - Core c: batch b = c // 4, rank r = c % 4.
  - QKV projection + attention are head-sharded: core r computes q heads
    4r..4r+3 and kv head r for ALL tokens of its batch.
  - Per-head AllGather re-shards y from head-split to token-split, overlapped
    with the next head's attention; the output projection then runs locally
    with the full contraction dim in 4 per-head passes (no all-reduce).

Layout tricks:
  - Host pre-transposes x, wqkv, wo so the kernel's matmuls need no on-device
    transposes (except tiny 128x128 PE transposes for V).
  - Rope's even/odd pair swap is a 128x128 permutation matmul in bf16
    (cheap; fp32 matmuls run at quarter rate).
  - Scores are computed transposed [kv, q]; exp is fused into the PSUM->SBUF
    eviction on the ScalarEngine, batched two kv-blocks per activation to
    amortize the ~300-cycle instruction overhead.
  - The softmax denominator is accumulated on the VectorEngine in bf16 (one
    pair-sum per exp batch, then a running add) and reduced across partitions
    with a single ones-vector matmul per q-chunk; the reciprocal is broadcast
    back to 128 partitions with a K=1 matmul, keeping the attention inner
    loop entirely off the GpSimd queue (whose collective waits would
    otherwise poison the pipeline).
  - RMSNorm reduces to a per-token scalar via a ones-vector matmul over the
    squared tile; the 1/sqrt(head_dim) score scale folds into the q-side
    scalar.
  - All big matmuls run in bf16 with f32 PSUM accumulation.
"""

import numpy as np
import ml_dtypes

import concourse.bass as bass
import concourse.bass_isa as bass_isa
import concourse.mybir as mybir
import concourse.tile as tile
from concourse import bacc
from concourse.bass_utils import run_bass_kernel_spmd

BF16 = mybir.dt.bfloat16
F32 = mybir.dt.float32

DIM = 2048
NH = 16
NKV = 4
HD = 128
EPS = 1e-5
N_CORES = 8
RG = [[0, 1, 2, 3], [4, 5, 6, 7]]  # per-batch tensor-parallel groups

HL = NH // NKV  # q heads per core (= GQA group size) = 4
EW = HL * HD + 2 * HD  # wqkv column-slice width per core = 768
NDT = DIM // 128  # contraction tiles = 16


def build_graph(S):
    """Build + compile the SPMD graph for sequence length S. Returns nc."""
    TPT = S // 4       # tokens per core after the gather (output rows per core)
    TCW = S // 4       # token chunk width for phase 1 (moving dim <= 512)
    NTT = S // TCW     # number of token chunks = 4
    QC = 512           # attention q-chunk width
    KB = 128           # kv block size
    NQC = S // QC      # q chunks per head
    NB = S // 128      # 128-token blocks (for V layout)
    AVDEPTH = 3        # kv-block pairs the AV matmuls trail the score matmuls

    nc = bacc.Bacc("TRN2", target_bir_lowering=False, debug=False,
                   num_devices=N_CORES)

    # ---- DRAM I/O ----
    xT_d = nc.dram_tensor("xT", [DIM, S], BF16, kind="ExternalInput")
    w_d = nc.dram_tensor("wslice", [DIM, EW], BF16, kind="ExternalInput")
    wo_d = nc.dram_tensor("woT", [DIM, DIM], BF16, kind="ExternalInput")
    cos_d = nc.dram_tensor("cosF", [128, S], F32, kind="ExternalInput")
    sin_d = nc.dram_tensor("sinF", [128, S], F32, kind="ExternalInput")
    swp_d = nc.dram_tensor("swapP", [128, 128], BF16, kind="ExternalInput")
    idn_d = nc.dram_tensor("ident", [128, 128], BF16, kind="ExternalInput")
    msk_d = nc.dram_tensor("masks", [KB, (QC // KB) * QC], BF16, kind="ExternalInput")
    qw_d = nc.dram_tensor("qw", [128, 1], F32, kind="ExternalInput")
    kw_d = nc.dram_tensor("kw", [128, 1], F32, kind="ExternalInput")
    out_d = nc.dram_tensor("out", [DIM, TPT], BF16, kind="ExternalOutput")

    with tile.TileContext(nc) as tc:
        with tc.tile_pool(name="const", bufs=1) as cpool, \
             tc.tile_pool(name="big", bufs=1) as bigpool, \
             tc.tile_pool(name="dram", bufs=1, space="DRAM") as dpool:

            # constants (gpsimd queue: keep the sync queue free for weights)
            swp = cpool.tile([128, 128], BF16, tag="swp")
            nc.gpsimd.dma_start(swp[:], swp_d[:])
            idn = cpool.tile([128, 128], BF16, tag="idn")
            nc.gpsimd.dma_start(idn[:], idn_d[:])
            msk = cpool.tile([KB, (QC // KB) * QC], BF16, tag="msk")
            nc.gpsimd.dma_start(msk[:], msk_d[:])
            qw = cpool.tile([128, 1], F32, tag="qw")
            nc.gpsimd.dma_start(qw[:], qw_d[:])
            kw = cpool.tile([128, 1], F32, tag="kw")
            nc.gpsimd.dma_start(kw[:], kw_d[:])
            ones = cpool.tile([128, 1], BF16, tag="ones")
            nc.vector.memset(ones[:], 1.0)
            onec = cpool.tile([1, 128], BF16, tag="onec")
            nc.vector.memset(onec[:], 1.0)
            b0 = cpool.tile([128, 1], F32, tag="b0")
            nc.vector.memset(b0[:], 0.0)
            bq = cpool.tile([1, 1], F32, tag="bq")
            nc.vector.memset(bq[:], float(HD * EPS))
            bk = cpool.tile([1, 1], F32, tag="bk")
            nc.vector.memset(bk[:], float(EPS))

            # long-lived activations
            qT = bigpool.tile([128, HL * S], BF16, tag="qT")
            kT = bigpool.tile([128, S], BF16, tag="kT")
            V = bigpool.tile([128, S], BF16, tag="V")   # [tok%128, blk*128+d]

            # tiny warm-up AllGather: pays the collective firmware's cold
            # start (~25us trigger-to-start + slow first gather) in the
            # shadow of phase-1 compute so the real per-head gathers fire
            # promptly
            wu_in = dpool.tile([128, 16], BF16, tag="wuin", name="wu_in")
            wu_out = dpool.tile([4 * 128, 16], BF16, tag="wuout",
                                name="wu_out")
            nc.gpsimd.collective_compute(
                "AllGather", mybir.AluOpType.bypass, replica_groups=RG,
                ins=[wu_in.opt()], outs=[wu_out.opt()])

            # ---------------- Phase 1: QKV + norm + rope ----------------
            with tc.tile_pool(name="wq", bufs=1) as wpool, \
                 tc.tile_pool(name="x", bufs=1) as xpool, \
                 tc.tile_pool(name="cs", bufs=2) as cspool, \
                 tc.tile_pool(name="scr", bufs=2) as scr, \
                 tc.tile_pool(name="smol", bufs=2) as smol, \
                 tc.tile_pool(name="p1", bufs=3, space="PSUM") as p1, \
                 tc.tile_pool(name="psw", bufs=2, space="PSUM") as psw, \
                 tc.tile_pool(name="pss", bufs=1, space="PSUM") as pss, \
                 tc.tile_pool(name="pvt", bufs=2, space="PSUM") as pvt:

                # full wqkv slice, staged once: [128, dt*EW + e]
                w_sb = wpool.tile([128, NDT * EW], BF16, tag="w")
                # full x, staged once with large contiguous DMAs
                xfull = xpool.tile([128, NDT * S], BF16, tag="x")

                def process_qk(ps, et, tt, cos_t, sin_t):
                    is_q = et < HL
                    # sum of squares over head_dim via ones-vector matmul
                    sqv = smol.tile([128, TCW], BF16, tag="sq2", name="sqv")
                    nc.scalar.activation(
                        sqv[:], ps[:],
                        mybir.ActivationFunctionType.Square, bias=b0[:])
                    ss = pss.tile([1, TCW], F32, tag="ss", name="ss")
                    nc.tensor.matmul(ss[:], ones[:], sqv[:],
                                     start=True, stop=True)
                    sq = smol.tile([1, TCW], F32, tag="sqs", name="sq")
                    if is_q:
                        # 1/sqrt(ss + HD*eps) folds the 1/sqrt(HD) score scale
                        nc.scalar.activation(
                            sq[:], ss[:],
                            mybir.ActivationFunctionType.Sqrt,
                            bias=bq[:], scale=1.0)
                    else:
                        nc.scalar.activation(
                            sq[:], ss[:],
                            mybir.ActivationFunctionType.Sqrt,
                            bias=bk[:], scale=1.0 / HD)
                    inv = smol.tile([1, TCW], F32, tag="inv", name="inv")
                    nc.vector.reciprocal_approx_fast(inv[:], sq[:])
                    invb = scr.tile([128, TCW], F32, tag="invb", name="invb")
                    nc.gpsimd.partition_broadcast(invb[:], inv[:])
                    qf = scr.tile([128, TCW], BF16, tag="qf", name="qf")
                    nc.scalar.mul(qf[:], ps[:], (qw if is_q else kw)[:])
                    # rope: pair swap via bf16 permutation matmul, sinF signed
                    sw = psw.tile([128, TCW], F32, tag="sw", name="sw")
                    nc.tensor.matmul(sw[:], swp[:], qf[:],
                                     start=True, stop=True)
                    t1 = scr.tile([128, TCW], F32, tag="t1", name="t1")
                    nc.vector.tensor_mul(t1[:], qf[:], cos_t[:])
                    t2 = scr.tile([128, TCW], F32, tag="t2", name="t2")
                    nc.vector.tensor_mul(t2[:], sw[:], sin_t[:])
                    nc.vector.tensor_add(t1[:], t1[:], t2[:])
                    dst = (qT[:, et * S + tt * TCW: et * S + tt * TCW + TCW]
                           if is_q else
                           kT[:, tt * TCW: tt * TCW + TCW])
                    nc.vector.tensor_mul(dst, t1[:], invb[:])

                def process_v(ps, tt):
                    vb = smol.tile([128, TCW], BF16, tag="vb", name="vb")
                    nc.scalar.copy(vb[:], ps[:])
                    for bb in range(TCW // 128):
                        tp = pvt.tile([128, 128], BF16, tag="tp", name="tp")
                        nc.tensor.transpose(
                            tp[:], vb[:, bb * 128:(bb + 1) * 128], idn[:])
                        blk = tt * (TCW // 128) + bb
                        nc.scalar.copy(V[:, blk * 128:(blk + 1) * 128], tp[:])

                pending = None  # (psum, et, tt, cos_t, sin_t)
                for dt in range(NDT):  # first token chunk + weights, need-order
                    nc.sync.dma_start(
                        w_sb[:, dt * EW:(dt + 1) * EW],
                        w_d[dt * 128:(dt + 1) * 128, :])
                    nc.scalar.dma_start(
                        xfull[:, dt * S:dt * S + TCW],
                        xT_d[dt * 128:(dt + 1) * 128, 0:TCW])
                for dt in range(NDT):  # bulk of x with large contiguous DMAs
                    eng = (nc.scalar, nc.gpsimd, nc.sync)[dt % 3]
                    eng.dma_start(
                        xfull[:, dt * S + TCW:(dt + 1) * S],
                        xT_d[dt * 128:(dt + 1) * 128, TCW:])
                for tt in range(NTT):
                    cos_t = cspool.tile([128, TCW], F32, tag="cos")
                    nc.sync.dma_start(cos_t[:], cos_d[:, tt * TCW:(tt + 1) * TCW])
                    sin_t = cspool.tile([128, TCW], F32, tag="sin")
                    nc.sync.dma_start(sin_t[:], sin_d[:, tt * TCW:(tt + 1) * TCW])

                    for et in range(HL + 2):
                        ps = p1.tile([128, TCW], F32, tag="ps")
                        for dt in range(NDT):
                            nc.tensor.matmul(
                                ps[:],
                                w_sb[:, dt * EW + et * 128:dt * EW + (et + 1) * 128],
                                xfull[:, dt * S + tt * TCW:
                                      dt * S + (tt + 1) * TCW],
                                start=(dt == 0), stop=(dt == NDT - 1),
                            )
                        # process the PREVIOUS tile now: its cross-engine
                        # waits overlap this tile's matmul group
                        if pending is not None:
                            pps, pet, ptt, pc, psn_ = pending
                            if pet < HL + 1:
                                process_qk(pps, pet, ptt, pc, psn_)
                            else:
                                process_v(pps, ptt)
                        pending = (ps, et, tt, cos_t, sin_t)
                pps, pet, ptt, pc, psn_ = pending
                if pet < HL + 1:
                    process_qk(pps, pet, ptt, pc, psn_)
                else:
                    process_v(pps, ptt)

            # ---------------- Phase 2: causal attention + outproj ----------------
            with tc.tile_pool(name="wo", bufs=1) as wopool, \
                 tc.tile_pool(name="part", bufs=1) as partpool, \
                 tc.tile_pool(name="yf", bufs=1) as yfpool, \
                 tc.tile_pool(name="yt", bufs=2) as ytpool, \
                 tc.tile_pool(name="acc", bufs=2) as accpool, \
                 tc.tile_pool(name="exp", bufs=8) as epool, \
                 tc.tile_pool(name="rs", bufs=2) as rspool, \
                 tc.tile_pool(name="ot", bufs=2) as otpool:

                part = partpool.tile([128, NDT * TPT], F32, tag="part")
                wo_h = [wopool.tile([128, 4 * S], BF16, tag=f"wo{h}",
                                    name=f"wo{h}")
                        for h in range(HL)]
                yf_h = [yfpool.tile([128, 4 * TPT], BF16, tag=f"yf{h}",
                                    name=f"yf{h}")
                        for h in range(HL)]
                ag_out = []  # (out_b, h) awaiting readback
                pid = nc.gpsimd.partition_id()
                roff = (pid % 4) * TPT

                def readback(h):
                    out_b = ag_out[h]
                    for r in range(4):
                        nc.gpsimd.dma_start(
                            yf_h[h][:, r * TPT:(r + 1) * TPT],
                            out_b[r * 128:(r + 1) * 128, bass.ds(roff, TPT)])

                attn_psum = tc.tile_pool(name="pa", bufs=2, space="PSUM")
                pa = attn_psum.__enter__()
                py_cm = tc.tile_pool(name="py", bufs=2, space="PSUM")
                py = py_cm.__enter__()
                pd_cm = tc.tile_pool(name="pd", bufs=1, space="PSUM")
                pd = pd_cm.__enter__()

                # prefetch ALL output-projection weight panels up front so the
                # per-head AllGathers face no competing DMA traffic
                for h in range(HL):
                    for j in range(4):
                        et = 4 * j + h
                        nc.sync.dma_start(
                            wo_h[h][:, j * S:(j + 1) * S],
                            wo_d[et * 128:(et + 1) * 128, :])

                for h in range(HL):
                    yT = ytpool.tile([128, S], BF16, tag="yT", name="yT")
                    in_b = dpool.tile([128, S], BF16, tag=f"agin{h}",
                                      name=f"agin{h}")
                    for qc in range(NQC):
                        nblk = 4 * (qc + 1)
                        nfull = 4 * qc
                        npair = nblk // 2
                        ps_y = py.tile([128, QC], F32, tag="y", name="ps_y")
                        acc = accpool.tile([128, QC], BF16, tag="acc", name="acc")
                        qsl = qT[:, h * S + qc * QC: h * S + (qc + 1) * QC]

                        pend_av = []  # (ex2, ga) pairs awaiting AV matmuls

                        def emit_av(ex2, ga):
                            for g, off in ((ga, 0), (ga + 1, QC)):
                                w0 = max(0, g - nfull) * KB
                                nc.tensor.matmul(
                                    ps_y[:, w0:QC],
                                    V[:, g * 128:(g + 1) * 128],
                                    ex2[:, off + w0: off + QC],
                                    start=(g == 0), stop=(g == nblk - 1))

                        for p in range(npair):
                            ga = 2 * p
                            pa2 = pa.tile([128, 2 * QC], F32, tag="s", name="pa2")
                            nc.tensor.matmul(
                                pa2[:, 0:QC],
                                kT[:, ga * KB:(ga + 1) * KB],
                                qsl, start=True, stop=True)
                            nc.tensor.matmul(
                                pa2[:, QC:2 * QC],
                                kT[:, (ga + 1) * KB:(ga + 2) * KB],
                                qsl, start=True, stop=True)
                            ex2 = epool.tile([128, 2 * QC], BF16, tag="e",
                                             name="ex2")
                            nc.scalar.activation(
                                ex2[:], pa2[:],
                                mybir.ActivationFunctionType.Exp, bias=b0[:])
                            if ga >= nfull:  # diagonal pair: causal mask
                                ta = ga - nfull
                                nc.vector.tensor_mul(
                                    ex2[:], ex2[:],
                                    msk[:, ta * QC:(ta + 2) * QC])
                            # denominator accumulation on DVE (bf16)
                            if p == 0:
                                nc.vector.tensor_add(
                                    acc[:], ex2[:, 0:QC], ex2[:, QC:2 * QC])
                            else:
                                ap = epool.tile([128, QC], BF16, tag="ap",
                                                name="accp")
                                nc.vector.tensor_add(
                                    ap[:], ex2[:, 0:QC], ex2[:, QC:2 * QC])
                                nc.vector.tensor_add(acc[:], acc[:], ap[:])
                            pend_av.append((ex2, ga))
                            if len(pend_av) > AVDEPTH:
                                emit_av(*pend_av.pop(0))
                        for args in pend_av:
                            emit_av(*args)

                        # denominator: ones-matmul over the bf16 accumulator;
                        # reciprocal broadcast back to 128 partitions via a
                        # K=1 matmul (keeps attention off the gpsimd queue)
                        den = pd.tile([1, QC], F32, tag="den", name="den")
                        nc.tensor.matmul(den[:], ones[:], acc[:],
                                         start=True, stop=True)
                        rec1 = rspool.tile([1, QC], F32, tag="rc1", name="rec1")
                        nc.vector.reciprocal_approx_fast(rec1[:], den[:])
                        rc16 = rspool.tile([1, QC], BF16, tag="rc6",
                                           name="rc16")
                        nc.vector.tensor_copy(rc16[:], rec1[:])
                        rec = pd.tile([128, QC], F32, tag="bc", name="rec")
                        nc.tensor.matmul(rec[:], onec[:], rc16[:],
                                         start=True, stop=True)
                        rsb = rspool.tile([128, QC], F32, tag="rsb",
                                          name="rsb")
                        nc.vector.tensor_copy(rsb[:], rec[:])
                        nc.vector.tensor_mul(
                            yT[:, qc * QC:(qc + 1) * QC], ps_y[:], rsb[:])
                        # stage this q-chunk's slice of the gather input now
                        nc.sync.dma_start(
                            in_b[:, qc * QC:(qc + 1) * QC],
                            yT[:, qc * QC:(qc + 1) * QC])

                    # per-head AllGather of y, overlapped with later heads
                    out_b = dpool.tile([4 * 128, S], BF16, tag=f"agout{h}",
                                       name=f"agout{h}")
                    nc.gpsimd.collective_compute(
                        "AllGather", mybir.AluOpType.bypass,
                        replica_groups=RG,
                        ins=[in_b.opt()], outs=[out_b.opt()])
                    ag_out.append(out_b)

                for h in range(HL):
                    readback(h)

                # release attention PSUM banks, open outproj pool
                pd_cm.__exit__(None, None, None)
                py_cm.__exit__(None, None, None)
                attn_psum.__exit__(None, None, None)
                po_cm = tc.tile_pool(name="po", bufs=4, space="PSUM")
                po = po_cm.__enter__()

                # ---- output projection: one pass per head, accumulated ----
                for h in range(HL):
                    last = (h == HL - 1)
                    for ot in range(NDT):
                        ps_o = po.tile([128, TPT], F32, tag="o", name="ps_o")
                        for j in range(4):
                            nc.tensor.matmul(
                                ps_o[:],
                                wo_h[h][:, j * S + ot * 128: j * S + ot * 128 + 128],
                                yf_h[h][:, j * TPT:(j + 1) * TPT],
                                start=(j == 0), stop=(j == 3))
                        psl = part[:, ot * TPT:(ot + 1) * TPT]
                        if h == 0:
                            nc.vector.tensor_copy(psl, ps_o[:])
                        elif not last:
                            nc.vector.tensor_add(psl, psl, ps_o[:])
                        else:
                            ott = otpool.tile([128, TPT], BF16, tag="ot",
                                              name="ott")
                            nc.vector.tensor_add(ott[:], ps_o[:], psl)
                            nc.sync.dma_start(
                                out_d[ot * 128:(ot + 1) * 128, :], ott[:])
                po_cm.__exit__(None, None, None)

    nc.compile()
    return nc


def make_in_maps(x, freqs_cis, wqkv, wo, q_norm_w, k_norm_w, S):
    """Host-side sharding / layout prep. Returns list of 8 input dicts."""
    bf = ml_dtypes.bfloat16
    QC = 512
    KB = 128

    # rope tables: [128, S]; row 2i & 2i+1 carry cos[t, i]; sin signed
    cos = np.asarray(freqs_cis[:S, :, 0], np.float32)   # [S, 64]
    sin = np.asarray(freqs_cis[:S, :, 1], np.float32)
    cosF = np.repeat(cos.T, 2, axis=0).astype(np.float32)      # [128, S]
    sinF = np.repeat(sin.T, 2, axis=0).astype(np.float32)
    sinF[0::2] *= -1.0
    cosF = np.ascontiguousarray(cosF)
    sinF = np.ascontiguousarray(sinF)

    swapP = np.zeros((128, 128), np.float32)
    for i in range(64):
        swapP[2 * i, 2 * i + 1] = 1.0
        swapP[2 * i + 1, 2 * i] = 1.0
    swapP = swapP.astype(bf)
    ident = np.eye(128, dtype=bf)

    # masks [KB, (QC//KB)*QC]: pattern t for the t-th kv block inside the
    # diagonal QC-region: allowed iff (t*KB + r) <= c
    r = np.arange(KB)[:, None]
    c = np.arange(QC)[None, :]
    pats = [((t * KB + r) <= c).astype(np.float32) for t in range(QC // KB)]
    masks = np.concatenate(pats, axis=1).astype(bf)

    qwv = np.asarray(q_norm_w, np.float32).reshape(128, 1)
    kwv = np.asarray(k_norm_w, np.float32).reshape(128, 1)

    woT = np.ascontiguousarray(np.asarray(wo, np.float32).T).astype(bf)

    xTb = []
    for b in range(2):
        xTb.append(np.ascontiguousarray(np.asarray(x[b], np.float32).T)
                   .astype(bf))

    wq = np.asarray(wqkv, np.float32)
    q_sz = NH * HD
    in_maps = []
    for c_id in range(N_CORES):
        b, rk = c_id // 4, c_id % 4
        rows = np.concatenate([
            wq[rk * HL * HD:(rk + 1) * HL * HD],          # 4 q heads
            wq[q_sz + rk * HD: q_sz + (rk + 1) * HD],     # k head
            wq[q_sz + NKV * HD + rk * HD:
               q_sz + NKV * HD + (rk + 1) * HD],          # v head
        ], axis=0)                                        # [768, 2048]
        wslice = np.ascontiguousarray(rows.T).astype(bf)  # [2048, 768]
        in_maps.append({
            "xT": xTb[b], "wslice": wslice, "woT": woT,
            "cosF": cosF, "sinF": sinF, "swapP": swapP,
            "ident": ident, "masks": masks,
            "qw": qwv, "kw": kwv,
        })
    return in_maps


_NC_CACHE = {}


def kernel(x, freqs_cis, mask, wqkv, wo, q_norm_w, k_norm_w):
    x = np.asarray(x)
    S = x.shape[1]
    if S not in _NC_CACHE:
        _NC_CACHE[S] = build_graph(S)
    nc = _NC_CACHE[S]
    in_maps = make_in_maps(x, freqs_cis, wqkv, wo, q_norm_w, k_norm_w, S)
    res = run_bass_kernel_spmd(nc, in_maps, core_ids=list(range(N_CORES)))
    TPT = S // 4
    out = np.empty((2, S, DIM), np.float32)
    for c_id in range(N_CORES):
        b, rk = c_id // 4, c_id % 4
        out[b, rk * TPT:(rk + 1) * TPT, :] = res.results[c_id]["out"].T.astype(np.float32)
    return out
